# revision 12
# baseline (speedup 1.0000x reference)
"""Trainium2 Bass kernel for nn_Arch7V3GraphEncoder (gnn_message_passing).

Strategy (graph/data parallel across 8 NeuronCores):
  - Canonical nodes are partitioned across the 8 cores. Because every edge is
    intra-subgraph (src and dst share e_sub) and subgraphs are node-local,
    each core runs the full 4-layer GIN stack on its shard with no
    communication; the final per-graph add-pool partials [32,128] are summed
    on the host.
  - Irregular gather/scatter is expressed as one-hot matmuls on the
    TensorEngine. Unlike the earlier revision (which shipped ~22 MB of
    host-built one-hot matrices per core and was transfer-bound), the host
    now sends only compact uint8 index arrays (~0.4 MB/core); the one-hot
    matrices are built on device:
      * column-wise one-hots (gather S, atom X, bond B): PE ones-broadcast of
        the index row into PSUM, then DVE is_equal against the partition iota;
      * row-wise one-hots (scatter D, graph G): DVE is_equal of a free-dim
        iota against a per-partition f32 index column.
    Invalid/empty slots carry index 200, which matches no iota lane and
    yields an all-zero one-hot column/row.
  - The (1+eps)*h term rides the MLP as an extra accumulating matmul; the
    bond-embedding matrix B is built once and staged through a DRAM scratch
    tile, streamed back per layer-group exactly like the old input path.
  - Pooling: subgraph masked-sum via a per-tile P1 one-hot matmul (built on
    device from the valid mask and a constant 128x16 pattern); softmax over
    log_probs on device; weighted reduce + canonical transposes + graph
    one-hot matmul produce the per-core [32,128] partial.
"""

import sys

sys.path.insert(0, "/opt/trn_rl_repo")

import numpy as np
import ml_dtypes

BF16 = ml_dtypes.bfloat16

# Problem constants (hardcoded per spec).
N_TOTAL = 4096
M_SUB = 4
K_NODES = 8
L_LAYERS = 4
H = 128
NUM_GRAPHS = 32
IN_CH = 119
EDGE_DIM = 8
S_ALL = N_TOTAL * M_SUB          # 16384 subgraphs
SK_ALL = S_ALL * K_NODES         # 131072 flat nodes
E_ALL = 12 * S_ALL               # 196608 edges
NCORES = 8
S_LOC = S_ALL // NCORES          # 2048 subgraphs / core
SK_LOC = SK_ALL // NCORES        # 16384 flat nodes / core
NT = SK_LOC // 128               # 128 tiles of 128 nodes
SG_T = 16                        # subgraphs per tile
NCAN_LOC = N_TOTAL // NCORES     # 512 canonical nodes / core
NQ = NCAN_LOC // 128             # 4 canonical chunks of 128
NG = NT // 4                     # 32 groups of 4 tiles (512 nodes)
EMPTY = 200                      # one-hot index that matches no lane


def _host_preprocess(inputs):
    """Compact integer index preprocessing -> small per-core uint8 arrays."""
    x_tokens = np.asarray(inputs["x_tokens"]).astype(np.int64)
    edge_tokens = np.asarray(inputs["edge_tokens"]).astype(np.int64)
    intra_ei = np.asarray(inputs["intra_ei"]).astype(np.int64)
    node_ids = np.asarray(inputs["node_ids"]).astype(np.int64)
    valid = np.asarray(inputs["valid"]).astype(bool)
    log_probs = np.asarray(inputs["log_probs"]).astype(np.float32)
    batch_graph = np.asarray(inputs["batch_graph"]).astype(np.int64)

    src, dst = intra_ei[0], intra_ei[1]
    e_sub = src // K_NODES
    assert np.array_equal(dst // K_NODES, e_sub), "edges must be intra-subgraph"

    core_of_e = e_sub // S_LOC
    tile_of_e = (e_sub % S_LOC) // SG_T
    key = core_of_e * NT + tile_of_e
    counts = np.bincount(key, minlength=NCORES * NT)
    e_cap = int(max(256, -(-counts.max() // 128) * 128))

    order = np.argsort(key, kind="stable")
    starts = np.zeros(NCORES * NT, dtype=np.int64)
    starts[1:] = np.cumsum(counts)[:-1]
    slot = np.empty(E_ALL, dtype=np.int64)
    slot[order] = np.arange(E_ALL) - starts[key[order]]

    ec = NT * e_cap
    nch = e_cap // 128
    nblk = ec // 128
    col = tile_of_e * e_cap + slot

    src_row = np.full((NCORES, 1, ec), EMPTY, dtype=np.uint8)
    tok_row = np.full((NCORES, 1, ec), EMPTY, dtype=np.uint8)
    dst_idx = np.full((NCORES, 128, nblk), EMPTY, dtype=np.uint8)
    src_row[core_of_e, 0, col] = np.where(valid[src], src % 128, EMPTY).astype(
        np.uint8
    )
    tok_row[core_of_e, 0, col] = edge_tokens.astype(np.uint8)
    dst_idx[core_of_e, slot % 128, tile_of_e * nch + slot // 128] = (
        dst % 128
    ).astype(np.uint8)

    j = np.arange(SK_ALL)
    j_core = j // SK_LOC
    j_loc = j % SK_LOC
    x_row = np.zeros((NCORES, 1, SK_LOC), dtype=np.uint8)
    x_row[j_core, 0, j_loc] = x_tokens.astype(np.uint8)

    vm = node_ids >= 0
    vmt = np.zeros((NCORES, 128, NT), dtype=np.uint8)
    vmt[j_core, j_loc % 128, j_loc // 128] = vm.astype(np.uint8)

    n = np.arange(N_TOTAL)
    bg = np.zeros((NCORES, 128, NQ), dtype=np.uint8)
    bg[n // NCAN_LOC, n % 128, (n % NCAN_LOC) // 128] = batch_graph.astype(np.uint8)

    p0 = np.zeros((128, SG_T), dtype=BF16)
    p0[np.arange(128), np.arange(128) // K_NODES] = np.asarray(1, dtype=BF16)

    lp = np.where(np.isfinite(log_probs), log_probs, 0.0).astype(np.float32)
    lp = lp.reshape(NCORES, 1, S_LOC)

    atom_emb = np.asarray(inputs["atom_emb"]).astype(np.float32)
    atom_pad = np.zeros((128, H), dtype=BF16)
    atom_pad[:IN_CH] = atom_emb.astype(BF16)

    b1 = np.asarray(inputs["mlp_b1"]).astype(np.float32)  # [L, H]
    b2 = np.asarray(inputs["mlp_b2"]).astype(np.float32)
    bvec = np.concatenate([b1.T, b2.T], axis=1)           # [H, 2L]
    sc = np.zeros((1, 8), dtype=np.float32)
    sc[0, :L_LAYERS] = np.asarray(inputs["eps"]).astype(np.float32)
    sc[0, L_LAYERS] = np.asarray(inputs["ht_alpha"]).astype(np.float32)[0]

    bf16_bytes = np.concatenate([
        p0.reshape(-1),
        atom_pad.reshape(-1),
        np.asarray(inputs["role_emb"]).astype(BF16).reshape(-1),
        np.asarray(inputs["bond_emb"]).astype(BF16).reshape(-1),
        np.asarray(inputs["mlp_w1"]).astype(BF16).reshape(-1),
        np.asarray(inputs["mlp_w2"]).astype(BF16).reshape(-1),
    ]).view(np.uint8)

    per_core = []
    for c in range(NCORES):
        u8_bytes = np.concatenate([
            src_row[c].reshape(-1), tok_row[c].reshape(-1),
            x_row[c].reshape(-1), dst_idx[c].reshape(-1),
            vmt[c].reshape(-1), bg[c].reshape(-1),
        ])
        f32_bytes = np.concatenate([
            lp[c].reshape(-1), bvec.reshape(-1), sc.reshape(-1)
        ]).astype(np.float32).view(np.uint8)
        blob = np.concatenate([u8_bytes, f32_bytes, bf16_bytes]).reshape(1, -1)
        per_core.append({"blob": np.ascontiguousarray(blob)})

    shared = {}
    return per_core, shared, e_cap


def _build_bass(e_cap, msg_bufs=4, pm_bufs=4, pz_bufs=1, pmlp_bufs=3, lay_bufs=2, repeat=1):
    import concourse.bass as bass
    import concourse.mybir as mybir
    from concourse import bacc
    from concourse.tile import TileContext

    f32 = mybir.dt.float32
    bf16 = mybir.dt.bfloat16
    u8 = mybir.dt.uint8
    i16 = mybir.dt.int16
    AF = mybir.ActivationFunctionType
    ALU = mybir.AluOpType
    AX = mybir.AxisListType

    ec = NT * e_cap
    nch = e_cap // 128  # edge chunks per tile
    nblk = ec // 128

    nc = bacc.Bacc("TRN2", target_bir_lowering=False, debug=False, num_devices=NCORES)

    def din(name, shape, dt):
        return nc.dram_tensor(name, shape, dt, kind="ExternalInput").ap()

    # single packed u8 blob; offsets in BYTES
    SRC_O = 0
    TOK_O = SRC_O + ec
    X_O = TOK_O + ec
    DST_O = X_O + SK_LOC
    VMT_O = DST_O + 128 * nblk
    BG_O = VMT_O + 128 * NT
    F32_BASE = BG_O + 128 * NQ
    LP_O = F32_BASE
    BVEC_O = LP_O + 4 * S_LOC
    SC_O = BVEC_O + 4 * H * 2 * L_LAYERS
    BF_BASE = SC_O + 4 * 8
    P0_O = BF_BASE
    ATOM_O = P0_O + 2 * 128 * SG_T
    ROLE_O = ATOM_O + 2 * 128 * H
    BOND_O = ROLE_O + 2 * 2 * H
    W1_O = BOND_O + 2 * 8 * H
    W2_O = W1_O + 2 * L_LAYERS * H * H
    BLOB_N = W2_O + 2 * L_LAYERS * H * H

    blob_d = din("blob", [1, BLOB_N], u8)

    def u8row(off, n):
        return blob_d[0:1, off : off + n]

    def u8m(off, p, c):
        return blob_d[0, off : off + p * c].rearrange("(p c) -> p c", p=p)

    def bfrow(off, n):
        return blob_d[0:1, off : off + 2 * n].bitcast(bf16)

    def bfm(off, p, c):
        return blob_d[0, off : off + 2 * p * c].bitcast(bf16).rearrange(
            "(p c) -> p c", p=p)

    def f32row(off, n):
        return blob_d[0:1, off : off + 4 * n].bitcast(f32)

    def f32m(off, p, c):
        return blob_d[0, off : off + 4 * p * c].bitcast(f32).rearrange(
            "(p c) -> p c", p=p)

    out_d = nc.dram_tensor("out", [NUM_GRAPHS, H], f32, kind="ExternalOutput").ap()

    with TileContext(nc) as tc:
        def _kernel_body():
            with (
                tc.tile_pool(name="persist", bufs=1) as pp,
                tc.tile_pool(name="dramp", bufs=1, space="DRAM") as dp,
            ):
                s_sb = pp.tile([128, ec], bf16, tag="s")
                hT = pp.tile([128, SK_LOC], bf16, tag="hT")
                h_nm = pp.tile([128, SK_LOC], bf16, tag="hnm")
                p1_sb = pp.tile([128, NT * SG_T], bf16, tag="p1")
                p0_sb = pp.tile([128, SG_T], bf16, tag="p0")
                g_sb = pp.tile([128, NQ * NUM_GRAPHS], bf16, tag="g")
                atom_sb = pp.tile([128, H], bf16, tag="atom")
                role_sb = pp.tile([2, H], bf16, tag="role")
                role1_sb = pp.tile([1, H], bf16, tag="role1")
                roleD = pp.tile([1, H], bf16, tag="roleD")
                role0T = pp.tile([128, 1], f32, tag="role0T")
                bond_sb = pp.tile([8, H], bf16, tag="bond")
                w1_sb = pp.tile([128, L_LAYERS * H], bf16, tag="w1")
                w2_sb = pp.tile([128, L_LAYERS * H], bf16, tag="w2")
                bvec_sb = pp.tile([128, 2 * L_LAYERS], f32, tag="bvec")
                eps_sb = pp.tile([1, L_LAYERS], f32, tag="eps")
                e1bc = pp.tile([128, L_LAYERS], f32, tag="e1bc")
                al_sb = pp.tile([1, 1], f32, tag="al")
                rc_sb = pp.tile([1, S_LOC], f32, tag="rc")
                lp_sb = pp.tile([1, S_LOC], f32, tag="lp")
                w_bc = pp.tile([128, S_LOC], f32, tag="wbc")
                rbc = pp.tile([128, S_LOC // M_SUB], f32, tag="rbc")
                id_sb = pp.tile([128, 128], bf16, tag="id")
                iota_f = pp.tile([128, 128], bf16, tag="iota_f")
                iota_p = pp.tile([128, 1], f32, tag="iota_p")
                ones_b = pp.tile([1, 128], bf16, tag="ones_b")
                ones_f = pp.tile([1, 128], f32, tag="ones_f")
                ones_c = pp.tile([128, 1], bf16, tag="ones_c")
                dst_f = pp.tile([128, nblk], f32, tag="dst_f")
                ir_row = pp.tile([1, 512], bf16, tag="ir_row")
                bscr = dp.tile([8, ec], bf16, tag="bscr")

                b1_sb = bvec_sb[:, 0:L_LAYERS]
                b2_sb = bvec_sb[:, L_LAYERS : 2 * L_LAYERS]

                # ---------------- constants / index prep ----------------
                nc.gpsimd.iota(iota_f, [[1, 128]], channel_multiplier=0,
                               allow_small_or_imprecise_dtypes=True)
                nc.gpsimd.iota(iota_p, [[1, 1]], channel_multiplier=1,
                               allow_small_or_imprecise_dtypes=True)
                nc.gpsimd.memset(ones_b, 1.0)
                nc.gpsimd.memset(ones_f, 1.0)
                nc.gpsimd.memset(ones_c, 1.0)

                nc.sync.dma_start(out=p0_sb, in_=bfm(P0_O, 128, SG_T))
                nc.sync.dma_start(out=atom_sb, in_=bfm(ATOM_O, 128, H))
                nc.sync.dma_start(out=role_sb, in_=bfm(ROLE_O, 2, H))
                nc.sync.dma_start(out=role1_sb, in_=bfrow(ROLE_O + 2 * H, H))
                nc.sync.dma_start(out=bond_sb, in_=bfm(BOND_O, 8, H))
                for l in range(L_LAYERS):
                    nc.sync.dma_start(out=w1_sb[:, l * H : (l + 1) * H], in_=bfm(W1_O + 2 * l * H * H, 128, H))
                    nc.sync.dma_start(out=w2_sb[:, l * H : (l + 1) * H], in_=bfm(W2_O + 2 * l * H * H, 128, H))
                nc.sync.dma_start(out=bvec_sb, in_=f32m(BVEC_O, H, 2 * L_LAYERS))
                nc.sync.dma_start(out=eps_sb, in_=f32row(SC_O, L_LAYERS))
                nc.sync.dma_start(out=al_sb, in_=f32row(SC_O + 4 * L_LAYERS, 1))
                nc.sync.dma_start(out=lp_sb, in_=f32row(LP_O, S_LOC))

                nc.vector.tensor_scalar(id_sb, iota_f, iota_p[:, 0:1], None,
                                        op0=ALU.is_equal)
                nc.vector.tensor_tensor(roleD, role1_sb, role_sb[0:1, :],
                                        ALU.subtract)

                with (
                    tc.tile_pool(name="prep_sb", bufs=3) as prep,
                    tc.tile_pool(name="prep_ps", bufs=1, space="PSUM") as prep_ps,
                ):
                    # role0T: transpose role_emb[0] into a [128,1] bias column
                    pr = prep_ps.tile([128, 2], bf16, tag="pr", bufs=1)
                    nc.tensor.transpose(pr, role_sb, id_sb[0:2, 0:2])
                    nc.vector.tensor_copy(role0T, pr[:, 0:1])

                    # (1+eps) broadcast [128, L]
                    pse = prep_ps.tile([128, L_LAYERS], f32, tag="pse", bufs=1)
                    nc.tensor.matmul(pse, lhsT=ones_f, rhs=eps_sb, start=True,
                                     stop=True)
                    nc.scalar.activation(e1bc, pse, AF.Copy, bias=1.0)

                    # isroot row: (j % 8 == 0), period 512
                    ir_i = prep.tile([1, 512], i16, tag="ir_i")
                    nc.gpsimd.iota(ir_i.rearrange("p (a b) -> p a b", b=K_NODES),
                                   [[0, 512 // K_NODES], [1, K_NODES]],
                                   channel_multiplier=0)
                    nc.vector.tensor_scalar(ir_row, ir_i, 0.0, None,
                                            op0=ALU.is_equal)

                    # graph one-hot [128, NQ*32] and dst_f conversion
                    bg_u8 = prep.tile([128, NQ], u8, tag="bg_u8", bufs=1)
                    nc.sync.dma_start(out=bg_u8, in_=u8m(BG_O, 128, NQ))
                    bg_f = prep.tile([128, NQ], f32, tag="bg_f", bufs=1)
                    nc.vector.tensor_copy(bg_f, bg_u8)
                    for q in range(NQ):
                        nc.vector.tensor_scalar(
                            g_sb[:, q * NUM_GRAPHS : (q + 1) * NUM_GRAPHS],
                            iota_f[:, 0:NUM_GRAPHS], bg_f[:, q : q + 1], None,
                            op0=ALU.is_equal)

                    dst_u8 = prep.tile([128, nblk], u8, tag="dst_u8", bufs=1)
                    nc.sync.dma_start(out=dst_u8, in_=u8m(DST_O, 128, nblk))
                    nc.vector.tensor_copy(dst_f, dst_u8)

                    # P1 pooling one-hot from valid mask
                    vmt_u8 = prep.tile([128, NT], u8, tag="vmt_u8", bufs=1)
                    nc.sync.dma_start(out=vmt_u8, in_=u8m(VMT_O, 128, NT))
                    vmt_f = prep.tile([128, NT], f32, tag="vmt_f", bufs=1)
                    nc.vector.tensor_copy(vmt_f, vmt_u8)
                    for t in range(NT):
                        nc.vector.tensor_scalar(
                            p1_sb[:, t * SG_T : (t + 1) * SG_T], p0_sb,
                            vmt_f[:, t : t + 1], None, op0=ALU.mult)

                    # S gather one-hot [128, ec] + B bond one-hot -> DRAM scratch
                    for c in range(ec // 512):
                        sl = slice(c * 512, (c + 1) * 512)
                        su = prep.tile([1, 512], u8, tag="su")
                        nc.sync.dma_start(out=su, in_=u8row(SRC_O + c * 512, 512))
                        sb_ = prep.tile([1, 512], bf16, tag="sb_")
                        nc.vector.tensor_copy(sb_, su)
                        pbc = prep_ps.tile([128, 512], f32, tag="pbc", bufs=3)
                        nc.tensor.matmul(pbc, lhsT=ones_b, rhs=sb_, start=True,
                                         stop=True)
                        nc.vector.tensor_scalar(s_sb[:, sl], pbc, iota_p[:, 0:1],
                                                None, op0=ALU.is_equal)

                        tu = prep.tile([1, 512], u8, tag="tu")
                        nc.sync.dma_start(out=tu, in_=u8row(TOK_O + c * 512, 512))
                        tb = prep.tile([1, 512], bf16, tag="tb")
                        nc.vector.tensor_copy(tb, tu)
                        pb8 = prep_ps.tile([8, 512], f32, tag="pb8", bufs=3)
                        nc.tensor.matmul(pb8, lhsT=ones_b[:, 0:8], rhs=tb,
                                         start=True, stop=True)
                        bt8 = prep.tile([8, 512], bf16, tag="bt8")
                        nc.vector.tensor_scalar(bt8, pb8, iota_p[0:8, 0:1], None,
                                                op0=ALU.is_equal)
                        nc.sync.dma_start(out=bscr[:, sl], in_=bt8)

                # ---------------- embed ----------------
                with (
                    tc.tile_pool(name="emb_sb", bufs=3) as ep,
                    tc.tile_pool(name="emb_ps", bufs=2, space="PSUM") as epp,
                    tc.tile_pool(name="emb_ps2", bufs=4, space="PSUM") as epp2,
                ):
                    for q in range(SK_LOC // 512):
                        sl = slice(q * 512, (q + 1) * 512)
                        xu = ep.tile([1, 512], u8, tag="xu")
                        nc.sync.dma_start(out=xu, in_=u8row(X_O + q * 512, 512))
                        xb = ep.tile([1, 512], bf16, tag="xb")
                        nc.vector.tensor_copy(xb, xu)
                        pxb = epp.tile([128, 512], f32, tag="pse")
                        nc.tensor.matmul(pxb, lhsT=ones_b, rhs=xb, start=True,
                                         stop=True)
                        xoh = ep.tile([128, 512], bf16, tag="xoh")
                        nc.vector.tensor_scalar(xoh, pxb, iota_p[:, 0:1], None,
                                                op0=ALU.is_equal)
                        ps = epp.tile([128, 512], f32, tag="pse")
                        nc.tensor.matmul(ps, lhsT=atom_sb, rhs=xoh, start=True,
                                         stop=False)
                        nc.tensor.matmul(ps, lhsT=roleD, rhs=ir_row, start=False,
                                         stop=True)
                        nc.scalar.activation(hT[:, sl], ps, AF.Identity, bias=role0T)

                    # softmax weights + subgraph-count reciprocals
                    for q in range(S_LOC // 512):
                        sl = slice(q * 512, (q + 1) * 512)
                        psc = epp.tile([1, 512], f32, tag="psc", bufs=2)
                        nc.tensor.matmul(psc, lhsT=ones_c, rhs=p1_sb[:, sl],
                                         start=True, stop=True)
                        cmx = ep.tile([1, 512], f32, tag="cmx", bufs=2)
                        nc.vector.tensor_scalar_max(cmx, psc, 1.0)
                        nc.vector.reciprocal(rc_sb[:, sl], cmx)

                    st = ep.tile([1, S_LOC], f32, tag="st", bufs=1)
                    nc.vector.tensor_scalar(
                        st, lp_sb, al_sb[:, 0:1], -1.0, op0=ALU.mult, op1=ALU.mult
                    )
                    et = ep.tile([1, S_LOC], f32, tag="et", bufs=1)
                    nc.scalar.activation(et, st, AF.Exp)
                    s4 = ep.tile([1, S_LOC // M_SUB], f32, tag="s4", bufs=1)
                    nc.vector.tensor_reduce(
                        s4, et.rearrange("p (a b) -> p a b", b=M_SUB), AX.X, ALU.add
                    )
                    r4 = ep.tile([1, S_LOC // M_SUB], f32, tag="r4", bufs=1)
                    nc.vector.reciprocal(r4, s4)
                    wr = ep.tile([1, S_LOC], f32, tag="wr", bufs=1)
                    nc.vector.tensor_tensor(wr, et, rc_sb, ALU.mult)
                    for q in range(S_LOC // 512):
                        pw = epp.tile([128, 512], f32, tag="pse")
                        nc.tensor.matmul(
                            pw, lhsT=ones_f, rhs=wr[:, q * 512 : (q + 1) * 512],
                            start=True, stop=True,
                        )
                        nc.vector.tensor_copy(w_bc[:, q * 512 : (q + 1) * 512], pw)
                    pw = epp.tile([128, 512], f32, tag="pse")
                    nc.tensor.matmul(pw, lhsT=ones_f, rhs=r4, start=True, stop=True)
                    nc.vector.tensor_copy(rbc, pw[:, : S_LOC // M_SUB])
                    for q in range(NT // 4):
                        pt = epp2.tile([128, 512], bf16, tag="pt", bufs=2)
                        for tt in range(4):
                            t = q * 4 + tt
                            nc.tensor.transpose(
                                pt[:, tt * 128 : (tt + 1) * 128],
                                hT[:, t * 128 : (t + 1) * 128], id_sb)
                        if q % 2 == 0:
                            nc.vector.tensor_copy(h_nm[:, q * 512 : (q + 1) * 512], pt)
                        else:
                            nc.scalar.activation(h_nm[:, q * 512 : (q + 1) * 512], pt, AF.Copy)

                # ---------------- layers ----------------
                with (
                    tc.tile_pool(name="lay_sb", bufs=lay_bufs) as lp_sbuf,
                    tc.tile_pool(name="msg_sb", bufs=msg_bufs) as mp,
                    tc.tile_pool(name="ps_m", bufs=pm_bufs, space="PSUM") as pm,
                    tc.tile_pool(name="ps_z", bufs=pz_bufs, space="PSUM") as pz,
                    tc.tile_pool(name="ps_mlp", bufs=pmlp_bufs, space="PSUM") as pmlp,
                ):
                    for l in range(L_LAYERS):
                        w1_l = w1_sb[:, l * H : (l + 1) * H]
                        w2_l = w2_sb[:, l * H : (l + 1) * H]
                        for g in range(NG):
                            dt_ = lp_sbuf.tile([128, 4 * e_cap], bf16, tag="d")
                            bt_ = lp_sbuf.tile([8, 4 * e_cap], bf16, tag="b")
                            for blk in range(4 * nch):
                                b = g * 4 * nch + blk
                                nc.vector.tensor_scalar(
                                    dt_[:, blk * 128 : (blk + 1) * 128], iota_f,
                                    dst_f[:, b : b + 1], None, op0=ALU.is_equal)
                            nc.sync.dma_start(
                                out=bt_, in_=bscr[:, g * 4 * e_cap : (g + 1) * 4 * e_cap]
                            )
                            psz = pz.tile([128, 512], f32, tag="z")
                            for tt in range(4):
                                t = g * 4 + tt
                                psm = pm.tile([128, e_cap], f32, tag="m")
                                for ch in range(nch):
                                    c0 = t * e_cap + ch * 128
                                    nc.tensor.matmul(
                                        psm[:, ch * 128 : (ch + 1) * 128],
                                        lhsT=s_sb[:, c0 : c0 + 128],
                                        rhs=h_nm[:, t * 128 : (t + 1) * 128],
                                        start=True,
                                        stop=False,
                                    )
                                    nc.tensor.matmul(
                                        psm[:, ch * 128 : (ch + 1) * 128],
                                        lhsT=bt_[:, tt * e_cap + ch * 128 : tt * e_cap + (ch + 1) * 128],
                                        rhs=bond_sb,
                                        start=False,
                                        stop=True,
                                    )
                                msg = mp.tile([128, e_cap], bf16, tag="msg")
                                if tt % 2 == 0:
                                    nc.scalar.activation(msg, psm, AF.Relu)
                                else:
                                    nc.vector.tensor_scalar_max(msg, psm, 0.0)
                                for ch in range(nch):
                                    nc.tensor.matmul(
                                        psz[:, tt * 128 : (tt + 1) * 128],
                                        lhsT=msg[:, ch * 128 : (ch + 1) * 128],
                                        rhs=dt_[:, tt * e_cap + ch * 128 : tt * e_cap + (ch + 1) * 128],
                                        start=(ch == 0),
                                        stop=(ch == nch - 1),
                                    )
                            gsl = slice(g * 512, (g + 1) * 512)
                            zin = mp.tile([128, 512], bf16, tag="aggr")
                            nc.vector.scalar_tensor_tensor(
                                zin, hT[:, gsl], e1bc[:, l : l + 1], psz,
                                op0=ALU.mult, op1=ALU.add,
                            )
                            psy = pmlp.tile([128, 512], f32, tag="y")
                            nc.tensor.matmul(psy, lhsT=w1_l, rhs=zin, start=True, stop=True)
                            y1 = mp.tile([128, 512], bf16, tag="y1")
                            nc.scalar.activation(y1, psy, AF.Relu, bias=b1_sb[:, l : l + 1])
                            psz2 = pmlp.tile([128, 512], f32, tag="y")
                            nc.tensor.matmul(psz2, lhsT=w2_l, rhs=y1, start=True, stop=True)
                            nc.scalar.activation(
                                hT[:, gsl], psz2, AF.Identity, bias=b2_sb[:, l : l + 1]
                            )
                            ptr = pmlp.tile([128, 512], bf16, tag="y")
                            for tt in range(4):
                                t0 = g * 4 + tt
                                nc.tensor.transpose(
                                    ptr[:, tt * 128 : (tt + 1) * 128],
                                    hT[:, t0 * 128 : (t0 + 1) * 128], id_sb)
                            if g % 2 == 0:
                                nc.vector.tensor_copy(h_nm[:, gsl], ptr)
                            else:
                                nc.scalar.activation(h_nm[:, gsl], ptr, AF.Copy)

                # ---------------- pooling ----------------
                with (
                    tc.tile_pool(name="po_sb", bufs=1) as po,
                    tc.tile_pool(name="po_big", bufs=1) as pob,
                    tc.tile_pool(name="ps_hs", bufs=1, space="PSUM") as phs,
                    tc.tile_pool(name="ps_sm", bufs=2, space="PSUM") as psm_p,
                    tc.tile_pool(name="ps_o", bufs=2, space="PSUM") as pso,
                ):
                    hs = phs.tile([128, S_LOC], f32, tag="hs")
                    for t in range(NT):
                        nc.tensor.matmul(
                            hs[:, t * SG_T : (t + 1) * SG_T],
                            lhsT=h_nm[:, t * 128 : (t + 1) * 128],
                            rhs=p1_sb[:, t * SG_T : (t + 1) * SG_T],
                            start=True,
                            stop=True,
                        )
                    wt = pob.tile([128, S_LOC], f32, tag="wt")
                    nc.vector.tensor_tensor(wt, hs, w_bc, ALU.mult)
                    ndT = pob.tile([128, NCAN_LOC], f32, tag="ndT")
                    nc.vector.tensor_reduce(
                        ndT,
                        wt.rearrange("p (a b) -> p a b", b=M_SUB),
                        AX.X,
                        ALU.add,
                    )
                    ndTb = pob.tile([128, NCAN_LOC], bf16, tag="ndTb")
                    nc.vector.tensor_tensor(ndTb, ndT, rbc, ALU.mult)
                    pout = pso.tile([NUM_GRAPHS, H], f32, tag="po")
                    for q in range(NQ):
                        ptq = psm_p.tile([128, 128], bf16, tag="pw")
                        nc.tensor.transpose(ptq, ndTb[:, q * 128 : (q + 1) * 128], id_sb)
                        nnm = po.tile([128, 128], bf16, tag="nnm")
                        nc.vector.tensor_copy(nnm, ptq)
                        nc.tensor.matmul(
                            pout,
                            lhsT=g_sb[:, q * NUM_GRAPHS : (q + 1) * NUM_GRAPHS],
                            rhs=nnm,
                            start=(q == 0),
                            stop=(q == NQ - 1),
                        )
                    outs = po.tile([NUM_GRAPHS, H], f32, tag="outs")
                    nc.scalar.activation(outs, pout, AF.Copy)
                    nc.sync.dma_start(out=out_d, in_=outs)

        if repeat > 1:
            with tc.For_i(0, repeat, 1) as _i:
                _kernel_body()
        else:
            _kernel_body()

    nc.finalize()
    return nc


_CACHE = {}


def _get_bass(e_cap):
    if e_cap not in _CACHE:
        _CACHE[e_cap] = _build_bass(e_cap)
    return _CACHE[e_cap]


def kernel(**inputs):
    from concourse.bass_utils import run_bass_kernel_spmd

    per_core, shared, e_cap = _host_preprocess(inputs)
    in_maps = [{**pc, **shared} for pc in per_core]
    nc = _get_bass(e_cap)
    res = run_bass_kernel_spmd(nc, in_maps, core_ids=list(range(NCORES)))
    out = np.zeros((NUM_GRAPHS, H), dtype=np.float32)
    for r in res.results:
        out += np.asarray(r["out"], dtype=np.float32)
    return out


# revision 13
# speedup vs baseline: 2.6796x; 2.6796x over previous
"""Trainium2 Bass kernel for nn_Arch7V3GraphEncoder (gnn_message_passing).

Strategy (graph/data parallel across 8 NeuronCores):
  - Canonical nodes are partitioned across the 8 cores. Because every edge is
    intra-subgraph (src and dst share e_sub) and subgraphs are node-local,
    each core runs the full 4-layer GIN stack on its shard with no
    communication; the final per-graph add-pool partials [32,128] are summed
    on the host.
  - Irregular gather/scatter is expressed as one-hot matmuls on the
    TensorEngine. Unlike the earlier revision (which shipped ~22 MB of
    host-built one-hot matrices per core and was transfer-bound), the host
    now sends only compact uint8 index arrays (~0.4 MB/core); the one-hot
    matrices are built on device:
      * column-wise one-hots (gather S, atom X, bond B): PE ones-broadcast of
        the index row into PSUM, then DVE is_equal against the partition iota;
      * row-wise one-hots (scatter D, graph G): DVE is_equal of a free-dim
        iota against a per-partition f32 index column.
    Invalid/empty slots carry index 200, which matches no iota lane and
    yields an all-zero one-hot column/row.
  - The (1+eps)*h term rides the MLP as an extra accumulating matmul; the
    bond-embedding matrix B is built once and staged through a DRAM scratch
    tile, streamed back per layer-group exactly like the old input path.
  - Pooling: subgraph masked-sum via a per-tile P1 one-hot matmul (built on
    device from the valid mask and a constant 128x16 pattern); softmax over
    log_probs on device; weighted reduce + canonical transposes + graph
    one-hot matmul produce the per-core [32,128] partial.
"""

import sys

sys.path.insert(0, "/opt/trn_rl_repo")

import numpy as np
import ml_dtypes

BF16 = ml_dtypes.bfloat16

# Problem constants (hardcoded per spec).
N_TOTAL = 4096
M_SUB = 4
K_NODES = 8
L_LAYERS = 4
H = 128
NUM_GRAPHS = 32
IN_CH = 119
EDGE_DIM = 8
S_ALL = N_TOTAL * M_SUB          # 16384 subgraphs
SK_ALL = S_ALL * K_NODES         # 131072 flat nodes
E_ALL = 12 * S_ALL               # 196608 edges
NCORES = 8
S_LOC = S_ALL // NCORES          # 2048 subgraphs / core
SK_LOC = SK_ALL // NCORES        # 16384 flat nodes / core
NT = SK_LOC // 128               # 128 tiles of 128 nodes
SG_T = 16                        # subgraphs per tile
NCAN_LOC = N_TOTAL // NCORES     # 512 canonical nodes / core
NQ = NCAN_LOC // 128             # 4 canonical chunks of 128
NG = NT // 4                     # 32 groups of 4 tiles (512 nodes)
EMPTY = 200                      # one-hot index that matches no lane


def _host_preprocess(inputs):
    """Compact integer index preprocessing -> small per-core uint8 arrays."""
    x_tokens = np.asarray(inputs["x_tokens"]).astype(np.int64)
    edge_tokens = np.asarray(inputs["edge_tokens"]).astype(np.int64)
    intra_ei = np.asarray(inputs["intra_ei"]).astype(np.int64)
    node_ids = np.asarray(inputs["node_ids"]).astype(np.int64)
    valid = np.asarray(inputs["valid"]).astype(bool)
    log_probs = np.asarray(inputs["log_probs"]).astype(np.float32)
    batch_graph = np.asarray(inputs["batch_graph"]).astype(np.int64)

    src, dst = intra_ei[0], intra_ei[1]
    e_sub = src // K_NODES
    assert np.array_equal(dst // K_NODES, e_sub), "edges must be intra-subgraph"

    core_of_e = e_sub // S_LOC
    tile_of_e = (e_sub % S_LOC) // SG_T
    key = core_of_e * NT + tile_of_e
    counts = np.bincount(key, minlength=NCORES * NT)
    e_cap = int(max(256, -(-counts.max() // 128) * 128))

    order = np.argsort(key, kind="stable")
    starts = np.zeros(NCORES * NT, dtype=np.int64)
    starts[1:] = np.cumsum(counts)[:-1]
    slot = np.empty(E_ALL, dtype=np.int64)
    slot[order] = np.arange(E_ALL) - starts[key[order]]

    ec = NT * e_cap
    nch = e_cap // 128
    nblk = ec // 128
    col = tile_of_e * e_cap + slot

    src_row = np.full((NCORES, 1, ec), EMPTY, dtype=np.uint8)
    tok_row = np.full((NCORES, 1, ec), EMPTY, dtype=np.uint8)
    dst_idx = np.full((NCORES, 128, nblk), EMPTY, dtype=np.uint8)
    src_row[core_of_e, 0, col] = np.where(valid[src], src % 128, EMPTY).astype(
        np.uint8
    )
    tok_row[core_of_e, 0, col] = edge_tokens.astype(np.uint8)
    dst_idx[core_of_e, slot % 128, tile_of_e * nch + slot // 128] = (
        dst % 128
    ).astype(np.uint8)

    j = np.arange(SK_ALL)
    j_core = j // SK_LOC
    j_loc = j % SK_LOC
    x_row = np.zeros((NCORES, 1, SK_LOC), dtype=np.uint8)
    x_row[j_core, 0, j_loc] = x_tokens.astype(np.uint8)

    vm = node_ids >= 0
    vmt = np.zeros((NCORES, 128, NT), dtype=np.uint8)
    vmt[j_core, j_loc % 128, j_loc // 128] = vm.astype(np.uint8)

    n = np.arange(N_TOTAL)
    bg = np.zeros((NCORES, 128, NQ), dtype=np.uint8)
    bg[n // NCAN_LOC, n % 128, (n % NCAN_LOC) // 128] = batch_graph.astype(np.uint8)

    p0 = np.zeros((128, SG_T), dtype=BF16)
    p0[np.arange(128), np.arange(128) // K_NODES] = np.asarray(1, dtype=BF16)

    lp = np.where(np.isfinite(log_probs), log_probs, 0.0).astype(np.float32)
    lp = lp.reshape(NCORES, 1, S_LOC)

    atom_emb = np.asarray(inputs["atom_emb"]).astype(np.float32)
    atom_pad = np.zeros((128, H), dtype=BF16)
    atom_pad[:IN_CH] = atom_emb.astype(BF16)

    b1 = np.asarray(inputs["mlp_b1"]).astype(np.float32)  # [L, H]
    b2 = np.asarray(inputs["mlp_b2"]).astype(np.float32)
    bvec = np.concatenate([b1.T, b2.T], axis=1)           # [H, 2L]
    sc = np.zeros((1, 8), dtype=np.float32)
    sc[0, :L_LAYERS] = np.asarray(inputs["eps"]).astype(np.float32)
    sc[0, L_LAYERS] = np.asarray(inputs["ht_alpha"]).astype(np.float32)[0]

    bf16_bytes = np.concatenate([
        p0.reshape(-1),
        atom_pad.reshape(-1),
        np.asarray(inputs["role_emb"]).astype(BF16).reshape(-1),
        np.asarray(inputs["bond_emb"]).astype(BF16).reshape(-1),
        np.asarray(inputs["mlp_w1"]).astype(BF16).reshape(-1),
        np.asarray(inputs["mlp_w2"]).astype(BF16).reshape(-1),
    ]).view(np.uint8)

    per_core = []
    for c in range(NCORES):
        u8_bytes = np.concatenate([
            src_row[c].reshape(-1), tok_row[c].reshape(-1),
            x_row[c].reshape(-1), dst_idx[c].reshape(-1),
            vmt[c].reshape(-1), bg[c].reshape(-1),
        ])
        f32_bytes = np.concatenate([
            lp[c].reshape(-1), bvec.reshape(-1), sc.reshape(-1)
        ]).astype(np.float32).view(np.uint8)
        blob = np.concatenate([u8_bytes, f32_bytes, bf16_bytes]).reshape(1, -1)
        per_core.append({"blob": np.ascontiguousarray(blob)})

    shared = {}
    return per_core, shared, e_cap


def _build_bass(e_cap, msg_bufs=4, pm_bufs=4, pz_bufs=1, pmlp_bufs=3, lay_bufs=2, repeat=1):
    import concourse.bass as bass
    import concourse.mybir as mybir
    from concourse import bacc
    from concourse.tile import TileContext

    f32 = mybir.dt.float32
    bf16 = mybir.dt.bfloat16
    u8 = mybir.dt.uint8
    i16 = mybir.dt.int16
    AF = mybir.ActivationFunctionType
    ALU = mybir.AluOpType
    AX = mybir.AxisListType

    ec = NT * e_cap
    nch = e_cap // 128  # edge chunks per tile
    nblk = ec // 128

    nc = bacc.Bacc("TRN2", target_bir_lowering=False, debug=False, num_devices=NCORES)

    def din(name, shape, dt):
        return nc.dram_tensor(name, shape, dt, kind="ExternalInput").ap()

    # single packed u8 blob; offsets in BYTES
    SRC_O = 0
    TOK_O = SRC_O + ec
    X_O = TOK_O + ec
    DST_O = X_O + SK_LOC
    VMT_O = DST_O + 128 * nblk
    BG_O = VMT_O + 128 * NT
    F32_BASE = BG_O + 128 * NQ
    LP_O = F32_BASE
    BVEC_O = LP_O + 4 * S_LOC
    SC_O = BVEC_O + 4 * H * 2 * L_LAYERS
    BF_BASE = SC_O + 4 * 8
    P0_O = BF_BASE
    ATOM_O = P0_O + 2 * 128 * SG_T
    ROLE_O = ATOM_O + 2 * 128 * H
    BOND_O = ROLE_O + 2 * 2 * H
    W1_O = BOND_O + 2 * 8 * H
    W2_O = W1_O + 2 * L_LAYERS * H * H
    BLOB_N = W2_O + 2 * L_LAYERS * H * H

    blob_d = din("blob", [1, BLOB_N], u8)

    def u8row(off, n):
        return blob_d[0:1, off : off + n]

    def u8m(off, p, c):
        return blob_d[0, off : off + p * c].rearrange("(p c) -> p c", p=p)

    def bfrow(off, n):
        return blob_d[0:1, off : off + 2 * n].bitcast(bf16)

    def bfm(off, p, c):
        return blob_d[0, off : off + 2 * p * c].bitcast(bf16).rearrange(
            "(p c) -> p c", p=p)

    def f32row(off, n):
        return blob_d[0:1, off : off + 4 * n].bitcast(f32)

    def f32m(off, p, c):
        return blob_d[0, off : off + 4 * p * c].bitcast(f32).rearrange(
            "(p c) -> p c", p=p)

    out_d = nc.dram_tensor("out", [NUM_GRAPHS, H], f32, kind="ExternalOutput").ap()

    with TileContext(nc) as tc:
        def _kernel_body():
            with (
                tc.tile_pool(name="persist", bufs=1) as pp,
                tc.tile_pool(name="dramp", bufs=1, space="DRAM") as dp,
            ):
                s_sb = pp.tile([128, ec], bf16, tag="s")
                hT = pp.tile([128, SK_LOC], bf16, tag="hT")
                h_nm = pp.tile([128, SK_LOC], bf16, tag="hnm")
                p1_sb = pp.tile([128, NT * SG_T], bf16, tag="p1")
                p0_sb = pp.tile([128, SG_T], bf16, tag="p0")
                g_sb = pp.tile([128, NQ * NUM_GRAPHS], bf16, tag="g")
                atom_sb = pp.tile([128, H], bf16, tag="atom")
                role_sb = pp.tile([2, H], bf16, tag="role")
                role1_sb = pp.tile([1, H], bf16, tag="role1")
                roleD = pp.tile([1, H], bf16, tag="roleD")
                role0T = pp.tile([128, 1], f32, tag="role0T")
                bond_sb = pp.tile([8, H], bf16, tag="bond")
                w1_sb = pp.tile([128, L_LAYERS * H], bf16, tag="w1")
                w2_sb = pp.tile([128, L_LAYERS * H], bf16, tag="w2")
                bvec_sb = pp.tile([128, 2 * L_LAYERS], f32, tag="bvec")
                eps_sb = pp.tile([1, L_LAYERS], f32, tag="eps")
                e1bc = pp.tile([128, L_LAYERS], f32, tag="e1bc")
                al_sb = pp.tile([1, 1], f32, tag="al")
                rc_sb = pp.tile([1, S_LOC], f32, tag="rc")
                lp_sb = pp.tile([1, S_LOC], f32, tag="lp")
                w_bc = pp.tile([128, S_LOC], f32, tag="wbc")
                rbc = pp.tile([128, S_LOC // M_SUB], f32, tag="rbc")
                id_sb = pp.tile([128, 128], bf16, tag="id")
                iota_f = pp.tile([128, 128], bf16, tag="iota_f")
                iota_p = pp.tile([128, 1], f32, tag="iota_p")
                ones_b = pp.tile([1, 128], bf16, tag="ones_b")
                ones_f = pp.tile([1, 128], f32, tag="ones_f")
                ones_c = pp.tile([128, 1], bf16, tag="ones_c")
                dst_f = pp.tile([128, nblk], f32, tag="dst_f")
                ir_row = pp.tile([1, 512], bf16, tag="ir_row")
                bscr = dp.tile([8, ec], bf16, tag="bscr")
                dscr = dp.tile([128, ec], bf16, tag="dscr")

                b1_sb = bvec_sb[:, 0:L_LAYERS]
                b2_sb = bvec_sb[:, L_LAYERS : 2 * L_LAYERS]

                # ---------------- constants / index prep ----------------
                nc.gpsimd.iota(iota_f, [[1, 128]], channel_multiplier=0,
                               allow_small_or_imprecise_dtypes=True)
                nc.gpsimd.iota(iota_p, [[1, 1]], channel_multiplier=1,
                               allow_small_or_imprecise_dtypes=True)
                nc.gpsimd.memset(ones_b, 1.0)
                nc.gpsimd.memset(ones_f, 1.0)
                nc.gpsimd.memset(ones_c, 1.0)

                nc.sync.dma_start(out=p0_sb, in_=bfm(P0_O, 128, SG_T))
                nc.sync.dma_start(out=atom_sb, in_=bfm(ATOM_O, 128, H))
                nc.sync.dma_start(out=role_sb, in_=bfm(ROLE_O, 2, H))
                nc.sync.dma_start(out=role1_sb, in_=bfrow(ROLE_O + 2 * H, H))
                nc.sync.dma_start(out=bond_sb, in_=bfm(BOND_O, 8, H))
                for l in range(L_LAYERS):
                    nc.sync.dma_start(out=w1_sb[:, l * H : (l + 1) * H], in_=bfm(W1_O + 2 * l * H * H, 128, H))
                    nc.sync.dma_start(out=w2_sb[:, l * H : (l + 1) * H], in_=bfm(W2_O + 2 * l * H * H, 128, H))
                nc.sync.dma_start(out=bvec_sb, in_=f32m(BVEC_O, H, 2 * L_LAYERS))
                nc.sync.dma_start(out=eps_sb, in_=f32row(SC_O, L_LAYERS))
                nc.sync.dma_start(out=al_sb, in_=f32row(SC_O + 4 * L_LAYERS, 1))
                nc.sync.dma_start(out=lp_sb, in_=f32row(LP_O, S_LOC))

                nc.vector.tensor_scalar(id_sb, iota_f, iota_p[:, 0:1], None,
                                        op0=ALU.is_equal)
                nc.vector.tensor_tensor(roleD, role1_sb, role_sb[0:1, :],
                                        ALU.subtract)

                with (
                    tc.tile_pool(name="prep_sb", bufs=3) as prep,
                    tc.tile_pool(name="prep_ps", bufs=1, space="PSUM") as prep_ps,
                ):
                    # role0T: transpose role_emb[0] into a [128,1] bias column
                    pr = prep_ps.tile([128, 2], bf16, tag="pr", bufs=1)
                    nc.tensor.transpose(pr, role_sb, id_sb[0:2, 0:2])
                    nc.vector.tensor_copy(role0T, pr[:, 0:1])

                    # (1+eps) broadcast [128, L]
                    pse = prep_ps.tile([128, L_LAYERS], f32, tag="pse", bufs=1)
                    nc.tensor.matmul(pse, lhsT=ones_f, rhs=eps_sb, start=True,
                                     stop=True)
                    nc.scalar.activation(e1bc, pse, AF.Copy, bias=1.0)

                    # isroot row: (j % 8 == 0), period 512
                    ir_i = prep.tile([1, 512], i16, tag="ir_i")
                    nc.gpsimd.iota(ir_i.rearrange("p (a b) -> p a b", b=K_NODES),
                                   [[0, 512 // K_NODES], [1, K_NODES]],
                                   channel_multiplier=0)
                    nc.vector.tensor_scalar(ir_row, ir_i, 0.0, None,
                                            op0=ALU.is_equal)

                    # graph one-hot [128, NQ*32] and dst_f conversion
                    bg_u8 = prep.tile([128, NQ], u8, tag="bg_u8", bufs=1)
                    nc.sync.dma_start(out=bg_u8, in_=u8m(BG_O, 128, NQ))
                    bg_f = prep.tile([128, NQ], f32, tag="bg_f", bufs=1)
                    nc.vector.tensor_copy(bg_f, bg_u8)
                    nc.vector.tensor_tensor(
                        g_sb.rearrange("p (q g) -> p q g", g=NUM_GRAPHS),
                        bg_f.rearrange("p (q x) -> p q x", x=1)
                            .broadcast_to([128, NQ, NUM_GRAPHS]),
                        iota_f[:, 0:NUM_GRAPHS]
                            .rearrange("p (x g) -> p x g", x=1)
                            .broadcast_to([128, NQ, NUM_GRAPHS]),
                        ALU.is_equal)

                    dst_u8 = prep.tile([128, nblk], u8, tag="dst_u8", bufs=1)
                    nc.sync.dma_start(out=dst_u8, in_=u8m(DST_O, 128, nblk))
                    nc.vector.tensor_copy(dst_f, dst_u8)
                    BPC = nblk // 8  # blocks per chunk
                    for k in range(8):
                        dchunk = prep.tile([128, BPC * 128], bf16, tag="dchunk",
                                           bufs=2)
                        nc.vector.tensor_tensor(
                            dchunk.rearrange("p (b c) -> p b c", c=128),
                            iota_f.rearrange("p (x c) -> p x c", x=1)
                                .broadcast_to([128, BPC, 128]),
                            dst_f[:, k * BPC : (k + 1) * BPC]
                                .rearrange("p (b x) -> p b x", x=1)
                                .broadcast_to([128, BPC, 128]),
                            ALU.is_equal)
                        nc.sync.dma_start(
                            out=dscr[:, k * BPC * 128 : (k + 1) * BPC * 128],
                            in_=dchunk)

                    # P1 pooling one-hot from valid mask
                    vmt_u8 = prep.tile([128, NT], u8, tag="vmt_u8", bufs=1)
                    nc.sync.dma_start(out=vmt_u8, in_=u8m(VMT_O, 128, NT))
                    vmt_f = prep.tile([128, NT], f32, tag="vmt_f", bufs=1)
                    nc.vector.tensor_copy(vmt_f, vmt_u8)
                    nc.vector.tensor_tensor(
                        p1_sb.rearrange("p (t s) -> p t s", s=SG_T),
                        vmt_f.rearrange("p (t x) -> p t x", x=1)
                            .broadcast_to([128, NT, SG_T]),
                        p0_sb.rearrange("p (x s) -> p x s", x=1)
                            .broadcast_to([128, NT, SG_T]),
                        ALU.mult)

                    # S gather one-hot [128, ec] + B bond one-hot -> DRAM scratch
                    for c in range(ec // 512):
                        sl = slice(c * 512, (c + 1) * 512)
                        su = prep.tile([1, 512], u8, tag="su")
                        nc.sync.dma_start(out=su, in_=u8row(SRC_O + c * 512, 512))
                        sb_ = prep.tile([1, 512], bf16, tag="sb_")
                        nc.vector.tensor_copy(sb_, su)
                        pbc = prep_ps.tile([128, 512], f32, tag="pbc", bufs=3)
                        nc.tensor.matmul(pbc, lhsT=ones_b, rhs=sb_, start=True,
                                         stop=True)
                        nc.vector.tensor_scalar(s_sb[:, sl], pbc, iota_p[:, 0:1],
                                                None, op0=ALU.is_equal)

                        tu = prep.tile([1, 512], u8, tag="tu")
                        nc.sync.dma_start(out=tu, in_=u8row(TOK_O + c * 512, 512))
                        tb = prep.tile([1, 512], bf16, tag="tb")
                        nc.vector.tensor_copy(tb, tu)
                        pb8 = prep_ps.tile([8, 512], f32, tag="pb8", bufs=3)
                        nc.tensor.matmul(pb8, lhsT=ones_b[:, 0:8], rhs=tb,
                                         start=True, stop=True)
                        bt8 = prep.tile([8, 512], bf16, tag="bt8")
                        nc.vector.tensor_scalar(bt8, pb8, iota_p[0:8, 0:1], None,
                                                op0=ALU.is_equal)
                        nc.sync.dma_start(out=bscr[:, sl], in_=bt8)

                # ---------------- embed ----------------
                with (
                    tc.tile_pool(name="emb_sb", bufs=3) as ep,
                    tc.tile_pool(name="emb_ps", bufs=2, space="PSUM") as epp,
                    tc.tile_pool(name="emb_ps2", bufs=4, space="PSUM") as epp2,
                ):
                    for q in range(SK_LOC // 512):
                        sl = slice(q * 512, (q + 1) * 512)
                        xu = ep.tile([1, 512], u8, tag="xu")
                        nc.sync.dma_start(out=xu, in_=u8row(X_O + q * 512, 512))
                        xb = ep.tile([1, 512], bf16, tag="xb")
                        nc.vector.tensor_copy(xb, xu)
                        pxb = epp.tile([128, 512], f32, tag="pse")
                        nc.tensor.matmul(pxb, lhsT=ones_b, rhs=xb, start=True,
                                         stop=True)
                        xoh = ep.tile([128, 512], bf16, tag="xoh")
                        nc.vector.tensor_scalar(xoh, pxb, iota_p[:, 0:1], None,
                                                op0=ALU.is_equal)
                        ps = epp.tile([128, 512], f32, tag="pse")
                        nc.tensor.matmul(ps, lhsT=atom_sb, rhs=xoh, start=True,
                                         stop=False)
                        nc.tensor.matmul(ps, lhsT=roleD, rhs=ir_row, start=False,
                                         stop=True)
                        nc.scalar.activation(hT[:, sl], ps, AF.Identity, bias=role0T)

                    # softmax weights + subgraph-count reciprocals
                    for q in range(S_LOC // 512):
                        sl = slice(q * 512, (q + 1) * 512)
                        psc = epp.tile([1, 512], f32, tag="psc", bufs=2)
                        nc.tensor.matmul(psc, lhsT=ones_c, rhs=p1_sb[:, sl],
                                         start=True, stop=True)
                        cmx = ep.tile([1, 512], f32, tag="cmx", bufs=2)
                        nc.vector.tensor_scalar_max(cmx, psc, 1.0)
                        nc.vector.reciprocal(rc_sb[:, sl], cmx)

                    st = ep.tile([1, S_LOC], f32, tag="st", bufs=1)
                    nc.vector.tensor_scalar(
                        st, lp_sb, al_sb[:, 0:1], -1.0, op0=ALU.mult, op1=ALU.mult
                    )
                    et = ep.tile([1, S_LOC], f32, tag="et", bufs=1)
                    nc.scalar.activation(et, st, AF.Exp)
                    s4 = ep.tile([1, S_LOC // M_SUB], f32, tag="s4", bufs=1)
                    nc.vector.tensor_reduce(
                        s4, et.rearrange("p (a b) -> p a b", b=M_SUB), AX.X, ALU.add
                    )
                    r4 = ep.tile([1, S_LOC // M_SUB], f32, tag="r4", bufs=1)
                    nc.vector.reciprocal(r4, s4)
                    wr = ep.tile([1, S_LOC], f32, tag="wr", bufs=1)
                    nc.vector.tensor_tensor(wr, et, rc_sb, ALU.mult)
                    for q in range(S_LOC // 512):
                        pw = epp.tile([128, 512], f32, tag="pse")
                        nc.tensor.matmul(
                            pw, lhsT=ones_f, rhs=wr[:, q * 512 : (q + 1) * 512],
                            start=True, stop=True,
                        )
                        nc.vector.tensor_copy(w_bc[:, q * 512 : (q + 1) * 512], pw)
                    pw = epp.tile([128, 512], f32, tag="pse")
                    nc.tensor.matmul(pw, lhsT=ones_f, rhs=r4, start=True, stop=True)
                    nc.vector.tensor_copy(rbc, pw[:, : S_LOC // M_SUB])
                    for q in range(NT // 4):
                        pt = epp2.tile([128, 512], bf16, tag="pt", bufs=2)
                        for tt in range(4):
                            t = q * 4 + tt
                            nc.tensor.transpose(
                                pt[:, tt * 128 : (tt + 1) * 128],
                                hT[:, t * 128 : (t + 1) * 128], id_sb)
                        if q % 2 == 0:
                            nc.vector.tensor_copy(h_nm[:, q * 512 : (q + 1) * 512], pt)
                        else:
                            nc.scalar.activation(h_nm[:, q * 512 : (q + 1) * 512], pt, AF.Copy)

                # ---------------- layers ----------------
                with (
                    tc.tile_pool(name="lay_sb", bufs=lay_bufs) as lp_sbuf,
                    tc.tile_pool(name="msg_sb", bufs=msg_bufs) as mp,
                    tc.tile_pool(name="ps_m", bufs=pm_bufs, space="PSUM") as pm,
                    tc.tile_pool(name="ps_z", bufs=pz_bufs, space="PSUM") as pz,
                    tc.tile_pool(name="ps_mlp", bufs=pmlp_bufs, space="PSUM") as pmlp,
                ):
                    for l in range(L_LAYERS):
                        w1_l = w1_sb[:, l * H : (l + 1) * H]
                        w2_l = w2_sb[:, l * H : (l + 1) * H]
                        for g in range(NG):
                            dt_ = lp_sbuf.tile([128, 4 * e_cap], bf16, tag="d")
                            bt_ = lp_sbuf.tile([8, 4 * e_cap], bf16, tag="b")
                            nc.sync.dma_start(
                                out=dt_, in_=dscr[:, g * 4 * e_cap : (g + 1) * 4 * e_cap]
                            )
                            nc.sync.dma_start(
                                out=bt_, in_=bscr[:, g * 4 * e_cap : (g + 1) * 4 * e_cap]
                            )
                            psz = pz.tile([128, 512], f32, tag="z")
                            for tt in range(4):
                                t = g * 4 + tt
                                psm = pm.tile([128, e_cap], f32, tag="m")
                                for ch in range(nch):
                                    c0 = t * e_cap + ch * 128
                                    nc.tensor.matmul(
                                        psm[:, ch * 128 : (ch + 1) * 128],
                                        lhsT=s_sb[:, c0 : c0 + 128],
                                        rhs=h_nm[:, t * 128 : (t + 1) * 128],
                                        start=True,
                                        stop=False,
                                    )
                                    nc.tensor.matmul(
                                        psm[:, ch * 128 : (ch + 1) * 128],
                                        lhsT=bt_[:, tt * e_cap + ch * 128 : tt * e_cap + (ch + 1) * 128],
                                        rhs=bond_sb,
                                        start=False,
                                        stop=True,
                                    )
                                msg = mp.tile([128, e_cap], bf16, tag="msg")
                                if tt % 2 == 0:
                                    nc.scalar.activation(msg, psm, AF.Relu)
                                else:
                                    nc.vector.tensor_scalar_max(msg, psm, 0.0)
                                for ch in range(nch):
                                    nc.tensor.matmul(
                                        psz[:, tt * 128 : (tt + 1) * 128],
                                        lhsT=msg[:, ch * 128 : (ch + 1) * 128],
                                        rhs=dt_[:, tt * e_cap + ch * 128 : tt * e_cap + (ch + 1) * 128],
                                        start=(ch == 0),
                                        stop=(ch == nch - 1),
                                    )
                            gsl = slice(g * 512, (g + 1) * 512)
                            zin = mp.tile([128, 512], bf16, tag="aggr")
                            nc.vector.scalar_tensor_tensor(
                                zin, hT[:, gsl], e1bc[:, l : l + 1], psz,
                                op0=ALU.mult, op1=ALU.add,
                            )
                            psy = pmlp.tile([128, 512], f32, tag="y")
                            nc.tensor.matmul(psy, lhsT=w1_l, rhs=zin, start=True, stop=True)
                            y1 = mp.tile([128, 512], bf16, tag="y1")
                            nc.scalar.activation(y1, psy, AF.Relu, bias=b1_sb[:, l : l + 1])
                            psz2 = pmlp.tile([128, 512], f32, tag="y")
                            nc.tensor.matmul(psz2, lhsT=w2_l, rhs=y1, start=True, stop=True)
                            nc.scalar.activation(
                                hT[:, gsl], psz2, AF.Identity, bias=b2_sb[:, l : l + 1]
                            )
                            ptr = pmlp.tile([128, 512], bf16, tag="y")
                            for tt in range(4):
                                t0 = g * 4 + tt
                                nc.tensor.transpose(
                                    ptr[:, tt * 128 : (tt + 1) * 128],
                                    hT[:, t0 * 128 : (t0 + 1) * 128], id_sb)
                            if g % 2 == 0:
                                nc.vector.tensor_copy(h_nm[:, gsl], ptr)
                            else:
                                nc.scalar.activation(h_nm[:, gsl], ptr, AF.Copy)

                # ---------------- pooling ----------------
                with (
                    tc.tile_pool(name="po_sb", bufs=1) as po,
                    tc.tile_pool(name="po_big", bufs=1) as pob,
                    tc.tile_pool(name="ps_hs", bufs=1, space="PSUM") as phs,
                    tc.tile_pool(name="ps_sm", bufs=2, space="PSUM") as psm_p,
                    tc.tile_pool(name="ps_o", bufs=2, space="PSUM") as pso,
                ):
                    hs = phs.tile([128, S_LOC], f32, tag="hs")
                    for t in range(NT):
                        nc.tensor.matmul(
                            hs[:, t * SG_T : (t + 1) * SG_T],
                            lhsT=h_nm[:, t * 128 : (t + 1) * 128],
                            rhs=p1_sb[:, t * SG_T : (t + 1) * SG_T],
                            start=True,
                            stop=True,
                        )
                    wt = pob.tile([128, S_LOC], f32, tag="wt")
                    nc.vector.tensor_tensor(wt, hs, w_bc, ALU.mult)
                    ndT = pob.tile([128, NCAN_LOC], f32, tag="ndT")
                    nc.vector.tensor_reduce(
                        ndT,
                        wt.rearrange("p (a b) -> p a b", b=M_SUB),
                        AX.X,
                        ALU.add,
                    )
                    ndTb = pob.tile([128, NCAN_LOC], bf16, tag="ndTb")
                    nc.vector.tensor_tensor(ndTb, ndT, rbc, ALU.mult)
                    pout = pso.tile([NUM_GRAPHS, H], f32, tag="po")
                    for q in range(NQ):
                        ptq = psm_p.tile([128, 128], bf16, tag="pw")
                        nc.tensor.transpose(ptq, ndTb[:, q * 128 : (q + 1) * 128], id_sb)
                        nnm = po.tile([128, 128], bf16, tag="nnm")
                        nc.vector.tensor_copy(nnm, ptq)
                        nc.tensor.matmul(
                            pout,
                            lhsT=g_sb[:, q * NUM_GRAPHS : (q + 1) * NUM_GRAPHS],
                            rhs=nnm,
                            start=(q == 0),
                            stop=(q == NQ - 1),
                        )
                    outs = po.tile([NUM_GRAPHS, H], f32, tag="outs")
                    nc.scalar.activation(outs, pout, AF.Copy)
                    nc.sync.dma_start(out=out_d, in_=outs)

        if repeat > 1:
            with tc.For_i(0, repeat, 1) as _i:
                _kernel_body()
        else:
            _kernel_body()

    nc.finalize()
    return nc


_CACHE = {}


def _get_bass(e_cap):
    if e_cap not in _CACHE:
        _CACHE[e_cap] = _build_bass(e_cap)
    return _CACHE[e_cap]


def kernel(**inputs):
    from concourse.bass_utils import run_bass_kernel_spmd

    per_core, shared, e_cap = _host_preprocess(inputs)
    in_maps = [{**pc, **shared} for pc in per_core]
    nc = _get_bass(e_cap)
    res = run_bass_kernel_spmd(nc, in_maps, core_ids=list(range(NCORES)))
    out = np.zeros((NUM_GRAPHS, H), dtype=np.float32)
    for r in res.results:
        out += np.asarray(r["out"], dtype=np.float32)
    return out


# revision 16
# speedup vs baseline: 4.4222x; 1.6503x over previous
"""Trainium2 Bass kernel for nn_Arch7V3GraphEncoder (gnn_message_passing).

Strategy (graph/data parallel across 8 NeuronCores):
  - Canonical nodes are partitioned across the 8 cores. Because every edge is
    intra-subgraph (src and dst share e_sub) and subgraphs are node-local,
    each core runs the full 4-layer GIN stack on its shard with no
    communication; the final per-graph add-pool partials [32,128] are summed
    on the host.
  - Irregular gather/scatter is expressed as one-hot matmuls on the
    TensorEngine. Unlike the earlier revision (which shipped ~22 MB of
    host-built one-hot matrices per core and was transfer-bound), the host
    now sends only compact uint8 index arrays (~0.4 MB/core); the one-hot
    matrices are built on device:
      * column-wise one-hots (gather S, atom X, bond B): PE ones-broadcast of
        the index row into PSUM, then DVE is_equal against the partition iota;
      * row-wise one-hots (scatter D, graph G): DVE is_equal of a free-dim
        iota against a per-partition f32 index column.
    Invalid/empty slots carry index 200, which matches no iota lane and
    yields an all-zero one-hot column/row.
  - The (1+eps)*h term rides the MLP as an extra accumulating matmul; the
    bond-embedding matrix B is built once and staged through a DRAM scratch
    tile, streamed back per layer-group exactly like the old input path.
  - Pooling: subgraph masked-sum via a per-tile P1 one-hot matmul (built on
    device from the valid mask and a constant 128x16 pattern); softmax over
    log_probs on device; weighted reduce + canonical transposes + graph
    one-hot matmul produce the per-core [32,128] partial.
"""

import sys

sys.path.insert(0, "/opt/trn_rl_repo")

import numpy as np
import ml_dtypes

BF16 = ml_dtypes.bfloat16

# Problem constants (hardcoded per spec).
N_TOTAL = 4096
M_SUB = 4
K_NODES = 8
L_LAYERS = 4
H = 128
NUM_GRAPHS = 32
IN_CH = 119
EDGE_DIM = 8
S_ALL = N_TOTAL * M_SUB          # 16384 subgraphs
SK_ALL = S_ALL * K_NODES         # 131072 flat nodes
E_ALL = 12 * S_ALL               # 196608 edges
NCORES = 8
S_LOC = S_ALL // NCORES          # 2048 subgraphs / core
SK_LOC = SK_ALL // NCORES        # 16384 flat nodes / core
NT = SK_LOC // 128               # 128 tiles of 128 nodes
SG_T = 16                        # subgraphs per tile
NCAN_LOC = N_TOTAL // NCORES     # 512 canonical nodes / core
NQ = NCAN_LOC // 128             # 4 canonical chunks of 128
NG = NT // 4                     # 32 groups of 4 tiles (512 nodes)
EMPTY = 200                      # one-hot index that matches no lane


def _host_preprocess(inputs):
    """Compact integer index preprocessing -> small per-core uint8 arrays."""
    x_tokens = np.asarray(inputs["x_tokens"]).astype(np.int64)
    edge_tokens = np.asarray(inputs["edge_tokens"]).astype(np.int64)
    intra_ei = np.asarray(inputs["intra_ei"]).astype(np.int64)
    node_ids = np.asarray(inputs["node_ids"]).astype(np.int64)
    valid = np.asarray(inputs["valid"]).astype(bool)
    log_probs = np.asarray(inputs["log_probs"]).astype(np.float32)
    batch_graph = np.asarray(inputs["batch_graph"]).astype(np.int64)

    src, dst = intra_ei[0], intra_ei[1]
    e_sub = src // K_NODES
    assert np.array_equal(dst // K_NODES, e_sub), "edges must be intra-subgraph"

    core_of_e = e_sub // S_LOC
    tile_of_e = (e_sub % S_LOC) // SG_T
    key = core_of_e * NT + tile_of_e
    counts = np.bincount(key, minlength=NCORES * NT)
    e_cap = int(max(256, -(-counts.max() // 128) * 128))

    order = np.argsort(key, kind="stable")
    starts = np.zeros(NCORES * NT, dtype=np.int64)
    starts[1:] = np.cumsum(counts)[:-1]
    slot = np.empty(E_ALL, dtype=np.int64)
    slot[order] = np.arange(E_ALL) - starts[key[order]]

    ec = NT * e_cap
    nch = e_cap // 128
    nblk = ec // 128
    col = tile_of_e * e_cap + slot

    src_row = np.full((NCORES, 1, ec), EMPTY, dtype=np.uint8)
    tok_row = np.full((NCORES, 1, ec), EMPTY, dtype=np.uint8)
    dst_idx = np.full((NCORES, 128, nblk), EMPTY, dtype=np.uint8)
    src_row[core_of_e, 0, col] = np.where(valid[src], src % 128, EMPTY).astype(
        np.uint8
    )
    tok_row[core_of_e, 0, col] = edge_tokens.astype(np.uint8)
    dst_idx[core_of_e, slot % 128, tile_of_e * nch + slot // 128] = (
        dst % 128
    ).astype(np.uint8)

    j = np.arange(SK_ALL)
    j_core = j // SK_LOC
    j_loc = j % SK_LOC
    x_row = np.zeros((NCORES, 1, SK_LOC), dtype=np.uint8)
    x_row[j_core, 0, j_loc] = x_tokens.astype(np.uint8)

    vm = node_ids >= 0
    vmt = np.zeros((NCORES, 128, NT), dtype=np.uint8)
    vmt[j_core, j_loc % 128, j_loc // 128] = vm.astype(np.uint8)

    n = np.arange(N_TOTAL)
    bg = np.zeros((NCORES, 128, NQ), dtype=np.uint8)
    bg[n // NCAN_LOC, n % 128, (n % NCAN_LOC) // 128] = batch_graph.astype(np.uint8)

    p0 = np.zeros((128, SG_T), dtype=BF16)
    p0[np.arange(128), np.arange(128) // K_NODES] = np.asarray(1, dtype=BF16)

    lp = np.where(np.isfinite(log_probs), log_probs, 0.0).astype(np.float32)
    lp = lp.reshape(NCORES, 1, S_LOC)

    atom_emb = np.asarray(inputs["atom_emb"]).astype(np.float32)
    atom_pad = np.zeros((128, H), dtype=BF16)
    atom_pad[:IN_CH] = atom_emb.astype(BF16)

    b1 = np.asarray(inputs["mlp_b1"]).astype(np.float32)  # [L, H]
    b2 = np.asarray(inputs["mlp_b2"]).astype(np.float32)
    bvec = np.concatenate([b1.T, b2.T], axis=1)           # [H, 2L]
    sc = np.zeros((1, 8), dtype=np.float32)
    sc[0, :L_LAYERS] = np.asarray(inputs["eps"]).astype(np.float32)
    sc[0, L_LAYERS] = np.asarray(inputs["ht_alpha"]).astype(np.float32)[0]

    bf16_bytes = np.concatenate([
        p0.reshape(-1),
        atom_pad.reshape(-1),
        np.asarray(inputs["role_emb"]).astype(BF16).reshape(-1),
        np.asarray(inputs["bond_emb"]).astype(BF16).reshape(-1),
        np.asarray(inputs["mlp_w1"]).astype(BF16).reshape(-1),
        np.asarray(inputs["mlp_w2"]).astype(BF16).reshape(-1),
    ]).view(np.uint8)

    per_core = []
    for c in range(NCORES):
        u8_bytes = np.concatenate([
            src_row[c].reshape(-1), tok_row[c].reshape(-1),
            x_row[c].reshape(-1), dst_idx[c].reshape(-1),
            vmt[c].reshape(-1), bg[c].reshape(-1),
        ])
        f32_bytes = np.concatenate([
            lp[c].reshape(-1), bvec.reshape(-1), sc.reshape(-1)
        ]).astype(np.float32).view(np.uint8)
        blob = np.concatenate([u8_bytes, f32_bytes, bf16_bytes]).reshape(1, -1)
        per_core.append({"blob": np.ascontiguousarray(blob)})

    shared = {}
    return per_core, shared, e_cap


def _build_bass(e_cap, msg_bufs=4, pm_bufs=4, pz_bufs=1, pmlp_bufs=3, lay_bufs=2, repeat=1):
    import concourse.bass as bass
    import concourse.mybir as mybir
    from concourse import bacc
    from concourse.tile import TileContext

    f32 = mybir.dt.float32
    bf16 = mybir.dt.bfloat16
    u8 = mybir.dt.uint8
    i16 = mybir.dt.int16
    AF = mybir.ActivationFunctionType
    ALU = mybir.AluOpType
    AX = mybir.AxisListType

    ec = NT * e_cap
    nch = e_cap // 128  # edge chunks per tile
    nblk = ec // 128

    nc = bacc.Bacc("TRN2", target_bir_lowering=False, debug=False, num_devices=NCORES)

    def din(name, shape, dt):
        return nc.dram_tensor(name, shape, dt, kind="ExternalInput").ap()

    # single packed u8 blob; offsets in BYTES
    SRC_O = 0
    TOK_O = SRC_O + ec
    X_O = TOK_O + ec
    DST_O = X_O + SK_LOC
    VMT_O = DST_O + 128 * nblk
    BG_O = VMT_O + 128 * NT
    F32_BASE = BG_O + 128 * NQ
    LP_O = F32_BASE
    BVEC_O = LP_O + 4 * S_LOC
    SC_O = BVEC_O + 4 * H * 2 * L_LAYERS
    BF_BASE = SC_O + 4 * 8
    P0_O = BF_BASE
    ATOM_O = P0_O + 2 * 128 * SG_T
    ROLE_O = ATOM_O + 2 * 128 * H
    BOND_O = ROLE_O + 2 * 2 * H
    W1_O = BOND_O + 2 * 8 * H
    W2_O = W1_O + 2 * L_LAYERS * H * H
    BLOB_N = W2_O + 2 * L_LAYERS * H * H

    blob_d = din("blob", [1, BLOB_N], u8)

    def u8row(off, n):
        return blob_d[0:1, off : off + n]

    def u8m(off, p, c):
        return blob_d[0, off : off + p * c].rearrange("(p c) -> p c", p=p)

    def bfrow(off, n):
        return blob_d[0:1, off : off + 2 * n].bitcast(bf16)

    def bfm(off, p, c):
        return blob_d[0, off : off + 2 * p * c].bitcast(bf16).rearrange(
            "(p c) -> p c", p=p)

    def f32row(off, n):
        return blob_d[0:1, off : off + 4 * n].bitcast(f32)

    def f32m(off, p, c):
        return blob_d[0, off : off + 4 * p * c].bitcast(f32).rearrange(
            "(p c) -> p c", p=p)

    out_d = nc.dram_tensor("out", [NUM_GRAPHS, H], f32, kind="ExternalOutput").ap()

    with TileContext(nc) as tc:
        def _kernel_body():
            with (
                tc.tile_pool(name="persist", bufs=1) as pp,
                tc.tile_pool(name="dramp", bufs=1, space="DRAM") as dp,
            ):
                s_sb = pp.tile([128, ec], bf16, tag="s")
                hT = pp.tile([128, SK_LOC], bf16, tag="hT")
                h_nm = pp.tile([128, SK_LOC], bf16, tag="hnm")
                p1_sb = pp.tile([128, NT * SG_T], bf16, tag="p1")
                p0_sb = pp.tile([128, SG_T], bf16, tag="p0")
                g_sb = pp.tile([128, NQ * NUM_GRAPHS], bf16, tag="g")
                atom_sb = pp.tile([128, H], bf16, tag="atom")
                role_sb = pp.tile([2, H], bf16, tag="role")
                role1_sb = pp.tile([1, H], bf16, tag="role1")
                roleD = pp.tile([1, H], bf16, tag="roleD")
                role0T = pp.tile([128, 1], f32, tag="role0T")
                bond_sb = pp.tile([8, H], bf16, tag="bond")
                w1_sb = pp.tile([128, L_LAYERS * H], bf16, tag="w1")
                w2_sb = pp.tile([128, L_LAYERS * H], bf16, tag="w2")
                bvec_sb = pp.tile([128, 2 * L_LAYERS], f32, tag="bvec")
                eps_sb = pp.tile([1, L_LAYERS], f32, tag="eps")
                e1bc = pp.tile([128, L_LAYERS], f32, tag="e1bc")
                al_sb = pp.tile([1, 1], f32, tag="al")
                rc_sb = pp.tile([1, S_LOC], f32, tag="rc")
                lp_sb = pp.tile([1, S_LOC], f32, tag="lp")
                w_bc = pp.tile([128, S_LOC], f32, tag="wbc")
                rbc = pp.tile([128, S_LOC // M_SUB], f32, tag="rbc")
                id_sb = pp.tile([128, 128], bf16, tag="id")
                iota_f = pp.tile([128, 128], bf16, tag="iota_f")
                iota_p = pp.tile([128, 1], f32, tag="iota_p")
                ones_f = pp.tile([1, 128], f32, tag="ones_f")
                ones_c = pp.tile([128, 1], bf16, tag="ones_c")
                dst_f = pp.tile([128, nblk], f32, tag="dst_f")
                ir_row = pp.tile([1, 512], bf16, tag="ir_row")
                bscr = dp.tile([8, ec], bf16, tag="bscr")
                dscr = dp.tile([128, ec], bf16, tag="dscr")

                b1_sb = bvec_sb[:, 0:L_LAYERS]
                b2_sb = bvec_sb[:, L_LAYERS : 2 * L_LAYERS]

                # ---------------- constants / index prep ----------------
                nc.gpsimd.iota(iota_f, [[1, 128]], channel_multiplier=0,
                               allow_small_or_imprecise_dtypes=True)
                nc.gpsimd.iota(iota_p, [[1, 1]], channel_multiplier=1,
                               allow_small_or_imprecise_dtypes=True)
                nc.gpsimd.memset(ones_f, 1.0)
                nc.gpsimd.memset(ones_c, 1.0)

                nc.sync.dma_start(out=p0_sb, in_=bfm(P0_O, 128, SG_T))
                nc.sync.dma_start(out=atom_sb, in_=bfm(ATOM_O, 128, H))
                nc.sync.dma_start(out=role_sb, in_=bfm(ROLE_O, 2, H))
                nc.sync.dma_start(out=role1_sb, in_=bfrow(ROLE_O + 2 * H, H))
                nc.sync.dma_start(out=bond_sb, in_=bfm(BOND_O, 8, H))
                for l in range(L_LAYERS):
                    nc.sync.dma_start(out=w1_sb[:, l * H : (l + 1) * H], in_=bfm(W1_O + 2 * l * H * H, 128, H))
                    nc.sync.dma_start(out=w2_sb[:, l * H : (l + 1) * H], in_=bfm(W2_O + 2 * l * H * H, 128, H))
                nc.sync.dma_start(out=bvec_sb, in_=f32m(BVEC_O, H, 2 * L_LAYERS))
                nc.sync.dma_start(out=eps_sb, in_=f32row(SC_O, L_LAYERS))
                nc.sync.dma_start(out=al_sb, in_=f32row(SC_O + 4 * L_LAYERS, 1))
                nc.sync.dma_start(out=lp_sb, in_=f32row(LP_O, S_LOC))

                nc.vector.tensor_scalar(id_sb, iota_f, iota_p[:, 0:1], None,
                                        op0=ALU.is_equal)
                nc.vector.tensor_tensor(roleD, role1_sb, role_sb[0:1, :],
                                        ALU.subtract)

                with (
                    tc.tile_pool(name="prep_sb", bufs=3) as prep,
                    tc.tile_pool(name="prep_ps", bufs=1, space="PSUM") as prep_ps,
                ):
                    # role0T: transpose role_emb[0] into a [128,1] bias column
                    pr = prep_ps.tile([128, 2], bf16, tag="pr", bufs=1)
                    nc.tensor.transpose(pr, role_sb, id_sb[0:2, 0:2])
                    nc.vector.tensor_copy(role0T, pr[:, 0:1])

                    # (1+eps) broadcast [128, L]
                    pse = prep_ps.tile([128, L_LAYERS], f32, tag="pse", bufs=1)
                    nc.tensor.matmul(pse, lhsT=ones_f, rhs=eps_sb, start=True,
                                     stop=True)
                    nc.scalar.activation(e1bc, pse, AF.Copy, bias=1.0)

                    # isroot row: (j % 8 == 0), period 512
                    ir_i = prep.tile([1, 512], i16, tag="ir_i")
                    nc.gpsimd.iota(ir_i.rearrange("p (a b) -> p a b", b=K_NODES),
                                   [[0, 512 // K_NODES], [1, K_NODES]],
                                   channel_multiplier=0)
                    nc.vector.tensor_scalar(ir_row, ir_i, 0.0, None,
                                            op0=ALU.is_equal)

                    # graph one-hot [128, NQ*32] and dst_f conversion
                    bg_u8 = prep.tile([128, NQ], u8, tag="bg_u8", bufs=1)
                    nc.sync.dma_start(out=bg_u8, in_=u8m(BG_O, 128, NQ))
                    bg_f = prep.tile([128, NQ], f32, tag="bg_f", bufs=1)
                    nc.vector.tensor_copy(bg_f, bg_u8)
                    nc.vector.tensor_tensor(
                        g_sb.rearrange("p (q g) -> p q g", g=NUM_GRAPHS),
                        bg_f.rearrange("p (q x) -> p q x", x=1)
                            .broadcast_to([128, NQ, NUM_GRAPHS]),
                        iota_f[:, 0:NUM_GRAPHS]
                            .rearrange("p (x g) -> p x g", x=1)
                            .broadcast_to([128, NQ, NUM_GRAPHS]),
                        ALU.is_equal)

                    dst_u8 = prep.tile([128, nblk], u8, tag="dst_u8", bufs=1)
                    nc.sync.dma_start(out=dst_u8, in_=u8m(DST_O, 128, nblk))
                    nc.vector.tensor_copy(dst_f, dst_u8)
                    BPC = nblk // 16  # blocks per chunk
                    for k in range(16):
                        dchunk = prep.tile([128, BPC * 128], bf16, tag="dchunk",
                                           bufs=2)
                        nc.vector.tensor_tensor(
                            dchunk.rearrange("p (b c) -> p b c", c=128),
                            iota_f.rearrange("p (x c) -> p x c", x=1)
                                .broadcast_to([128, BPC, 128]),
                            dst_f[:, k * BPC : (k + 1) * BPC]
                                .rearrange("p (b x) -> p b x", x=1)
                                .broadcast_to([128, BPC, 128]),
                            ALU.is_equal)
                        nc.sync.dma_start(
                            out=dscr[:, k * BPC * 128 : (k + 1) * BPC * 128],
                            in_=dchunk)

                    # P1 pooling one-hot from valid mask
                    vmt_u8 = prep.tile([128, NT], u8, tag="vmt_u8", bufs=1)
                    nc.sync.dma_start(out=vmt_u8, in_=u8m(VMT_O, 128, NT))
                    vmt_f = prep.tile([128, NT], f32, tag="vmt_f", bufs=1)
                    nc.vector.tensor_copy(vmt_f, vmt_u8)
                    nc.vector.tensor_tensor(
                        p1_sb.rearrange("p (t s) -> p t s", s=SG_T),
                        vmt_f.rearrange("p (t x) -> p t x", x=1)
                            .broadcast_to([128, NT, SG_T]),
                        p0_sb.rearrange("p (x s) -> p x s", x=1)
                            .broadcast_to([128, NT, SG_T]),
                        ALU.mult)

                    # S gather one-hot [128, ec] + B bond one-hot -> DRAM scratch.
                    # The u8 index rows are partition-broadcast by DMA, then a
                    # single is_equal against the partition iota per chunk.
                    SCH = 4096
                    for k in range(ec // SCH):
                        sl = slice(k * SCH, (k + 1) * SCH)
                        sstage = prep.tile([128, SCH], u8, tag="sstage", bufs=2)
                        nc.sync.dma_start(
                            out=sstage,
                            in_=u8row(SRC_O + k * SCH, SCH).broadcast_to([128, SCH]))
                        nc.vector.tensor_scalar(s_sb[:, sl], sstage,
                                                iota_p[:, 0:1], None,
                                                op0=ALU.is_equal)

                        tstage = prep.tile([8, SCH], u8, tag="tstage", bufs=2)
                        nc.sync.dma_start(
                            out=tstage,
                            in_=u8row(TOK_O + k * SCH, SCH).broadcast_to([8, SCH]))
                        bt8 = prep.tile([8, SCH], bf16, tag="bt8", bufs=1)
                        nc.vector.tensor_scalar(bt8, tstage, iota_p[0:8, 0:1],
                                                None, op0=ALU.is_equal)
                        nc.sync.dma_start(out=bscr[:, sl], in_=bt8)

                # ---------------- embed ----------------
                with (
                    tc.tile_pool(name="emb_sb", bufs=3) as ep,
                    tc.tile_pool(name="emb_ps", bufs=2, space="PSUM") as epp,
                    tc.tile_pool(name="emb_ps2", bufs=4, space="PSUM") as epp2,
                ):
                    for q in range(SK_LOC // 512):
                        sl = slice(q * 512, (q + 1) * 512)
                        xstage = ep.tile([128, 512], u8, tag="xstage")
                        nc.sync.dma_start(
                            out=xstage,
                            in_=u8row(X_O + q * 512, 512).broadcast_to([128, 512]))
                        xoh = ep.tile([128, 512], bf16, tag="xoh")
                        nc.vector.tensor_scalar(xoh, xstage, iota_p[:, 0:1], None,
                                                op0=ALU.is_equal)
                        ps = epp.tile([128, 512], f32, tag="pse")
                        nc.tensor.matmul(ps, lhsT=atom_sb, rhs=xoh, start=True,
                                         stop=False)
                        nc.tensor.matmul(ps, lhsT=roleD, rhs=ir_row, start=False,
                                         stop=True)
                        nc.scalar.activation(hT[:, sl], ps, AF.Identity, bias=role0T)

                    # softmax weights + subgraph-count reciprocals
                    for q in range(S_LOC // 512):
                        sl = slice(q * 512, (q + 1) * 512)
                        psc = epp.tile([1, 512], f32, tag="psc", bufs=2)
                        nc.tensor.matmul(psc, lhsT=ones_c, rhs=p1_sb[:, sl],
                                         start=True, stop=True)
                        cmx = ep.tile([1, 512], f32, tag="cmx", bufs=2)
                        nc.vector.tensor_scalar_max(cmx, psc, 1.0)
                        nc.vector.reciprocal(rc_sb[:, sl], cmx)

                    st = ep.tile([1, S_LOC], f32, tag="st", bufs=1)
                    nc.vector.tensor_scalar(
                        st, lp_sb, al_sb[:, 0:1], -1.0, op0=ALU.mult, op1=ALU.mult
                    )
                    et = ep.tile([1, S_LOC], f32, tag="et", bufs=1)
                    nc.scalar.activation(et, st, AF.Exp)
                    s4 = ep.tile([1, S_LOC // M_SUB], f32, tag="s4", bufs=1)
                    nc.vector.tensor_reduce(
                        s4, et.rearrange("p (a b) -> p a b", b=M_SUB), AX.X, ALU.add
                    )
                    r4 = ep.tile([1, S_LOC // M_SUB], f32, tag="r4", bufs=1)
                    nc.vector.reciprocal(r4, s4)
                    wr = ep.tile([1, S_LOC], f32, tag="wr", bufs=1)
                    nc.vector.tensor_tensor(wr, et, rc_sb, ALU.mult)
                    for q in range(S_LOC // 512):
                        pw = epp.tile([128, 512], f32, tag="pse")
                        nc.tensor.matmul(
                            pw, lhsT=ones_f, rhs=wr[:, q * 512 : (q + 1) * 512],
                            start=True, stop=True,
                        )
                        nc.vector.tensor_copy(w_bc[:, q * 512 : (q + 1) * 512], pw)
                    pw = epp.tile([128, 512], f32, tag="pse")
                    nc.tensor.matmul(pw, lhsT=ones_f, rhs=r4, start=True, stop=True)
                    nc.vector.tensor_copy(rbc, pw[:, : S_LOC // M_SUB])
                    for q in range(NT // 4):
                        pt = epp2.tile([128, 512], bf16, tag="pt", bufs=2)
                        for tt in range(4):
                            t = q * 4 + tt
                            nc.tensor.transpose(
                                pt[:, tt * 128 : (tt + 1) * 128],
                                hT[:, t * 128 : (t + 1) * 128], id_sb)
                        if q % 2 == 0:
                            nc.vector.tensor_copy(h_nm[:, q * 512 : (q + 1) * 512], pt)
                        else:
                            nc.scalar.activation(h_nm[:, q * 512 : (q + 1) * 512], pt, AF.Copy)

                # ---------------- layers ----------------
                with (
                    tc.tile_pool(name="lay_sb", bufs=lay_bufs) as lp_sbuf,
                    tc.tile_pool(name="msg_sb", bufs=msg_bufs) as mp,
                    tc.tile_pool(name="ps_m", bufs=pm_bufs, space="PSUM") as pm,
                    tc.tile_pool(name="ps_z", bufs=pz_bufs, space="PSUM") as pz,
                    tc.tile_pool(name="ps_mlp", bufs=pmlp_bufs, space="PSUM") as pmlp,
                ):
                    for l in range(L_LAYERS):
                        w1_l = w1_sb[:, l * H : (l + 1) * H]
                        w2_l = w2_sb[:, l * H : (l + 1) * H]
                        for g in range(NG):
                            dt_ = lp_sbuf.tile([128, 4 * e_cap], bf16, tag="d")
                            bt_ = lp_sbuf.tile([8, 4 * e_cap], bf16, tag="b")
                            nc.sync.dma_start(
                                out=dt_, in_=dscr[:, g * 4 * e_cap : (g + 1) * 4 * e_cap]
                            )
                            nc.sync.dma_start(
                                out=bt_, in_=bscr[:, g * 4 * e_cap : (g + 1) * 4 * e_cap]
                            )
                            psz = pz.tile([128, 512], f32, tag="z")
                            for tt in range(4):
                                t = g * 4 + tt
                                psm = pm.tile([128, e_cap], f32, tag="m")
                                for ch in range(nch):
                                    c0 = t * e_cap + ch * 128
                                    nc.tensor.matmul(
                                        psm[:, ch * 128 : (ch + 1) * 128],
                                        lhsT=s_sb[:, c0 : c0 + 128],
                                        rhs=h_nm[:, t * 128 : (t + 1) * 128],
                                        start=True,
                                        stop=False,
                                    )
                                    nc.tensor.matmul(
                                        psm[:, ch * 128 : (ch + 1) * 128],
                                        lhsT=bt_[:, tt * e_cap + ch * 128 : tt * e_cap + (ch + 1) * 128],
                                        rhs=bond_sb,
                                        start=False,
                                        stop=True,
                                    )
                                msg = mp.tile([128, e_cap], bf16, tag="msg")
                                if tt % 2 == 0:
                                    nc.scalar.activation(msg, psm, AF.Relu)
                                else:
                                    nc.vector.tensor_scalar_max(msg, psm, 0.0)
                                for ch in range(nch):
                                    nc.tensor.matmul(
                                        psz[:, tt * 128 : (tt + 1) * 128],
                                        lhsT=msg[:, ch * 128 : (ch + 1) * 128],
                                        rhs=dt_[:, tt * e_cap + ch * 128 : tt * e_cap + (ch + 1) * 128],
                                        start=(ch == 0),
                                        stop=(ch == nch - 1),
                                    )
                            gsl = slice(g * 512, (g + 1) * 512)
                            zin = mp.tile([128, 512], bf16, tag="aggr")
                            nc.vector.scalar_tensor_tensor(
                                zin, hT[:, gsl], e1bc[:, l : l + 1], psz,
                                op0=ALU.mult, op1=ALU.add,
                            )
                            psy = pmlp.tile([128, 512], f32, tag="y")
                            nc.tensor.matmul(psy, lhsT=w1_l, rhs=zin, start=True, stop=True)
                            y1 = mp.tile([128, 512], bf16, tag="y1")
                            nc.scalar.activation(y1, psy, AF.Relu, bias=b1_sb[:, l : l + 1])
                            psz2 = pmlp.tile([128, 512], f32, tag="y")
                            nc.tensor.matmul(psz2, lhsT=w2_l, rhs=y1, start=True, stop=True)
                            nc.scalar.activation(
                                hT[:, gsl], psz2, AF.Identity, bias=b2_sb[:, l : l + 1]
                            )
                            ptr = pmlp.tile([128, 512], bf16, tag="y")
                            for tt in range(4):
                                t0 = g * 4 + tt
                                nc.tensor.transpose(
                                    ptr[:, tt * 128 : (tt + 1) * 128],
                                    hT[:, t0 * 128 : (t0 + 1) * 128], id_sb)
                            if g % 2 == 0:
                                nc.vector.tensor_copy(h_nm[:, gsl], ptr)
                            else:
                                nc.scalar.activation(h_nm[:, gsl], ptr, AF.Copy)

                # ---------------- pooling ----------------
                with (
                    tc.tile_pool(name="po_sb", bufs=1) as po,
                    tc.tile_pool(name="po_big", bufs=1) as pob,
                    tc.tile_pool(name="ps_hs", bufs=1, space="PSUM") as phs,
                    tc.tile_pool(name="ps_sm", bufs=2, space="PSUM") as psm_p,
                    tc.tile_pool(name="ps_o", bufs=2, space="PSUM") as pso,
                ):
                    hs = phs.tile([128, S_LOC], f32, tag="hs")
                    for t in range(NT):
                        nc.tensor.matmul(
                            hs[:, t * SG_T : (t + 1) * SG_T],
                            lhsT=h_nm[:, t * 128 : (t + 1) * 128],
                            rhs=p1_sb[:, t * SG_T : (t + 1) * SG_T],
                            start=True,
                            stop=True,
                        )
                    wt = pob.tile([128, S_LOC], f32, tag="wt")
                    nc.vector.tensor_tensor(wt, hs, w_bc, ALU.mult)
                    ndT = pob.tile([128, NCAN_LOC], f32, tag="ndT")
                    nc.vector.tensor_reduce(
                        ndT,
                        wt.rearrange("p (a b) -> p a b", b=M_SUB),
                        AX.X,
                        ALU.add,
                    )
                    ndTb = pob.tile([128, NCAN_LOC], bf16, tag="ndTb")
                    nc.vector.tensor_tensor(ndTb, ndT, rbc, ALU.mult)
                    pout = pso.tile([NUM_GRAPHS, H], f32, tag="po")
                    for q in range(NQ):
                        ptq = psm_p.tile([128, 128], bf16, tag="pw")
                        nc.tensor.transpose(ptq, ndTb[:, q * 128 : (q + 1) * 128], id_sb)
                        nnm = po.tile([128, 128], bf16, tag="nnm")
                        nc.vector.tensor_copy(nnm, ptq)
                        nc.tensor.matmul(
                            pout,
                            lhsT=g_sb[:, q * NUM_GRAPHS : (q + 1) * NUM_GRAPHS],
                            rhs=nnm,
                            start=(q == 0),
                            stop=(q == NQ - 1),
                        )
                    outs = po.tile([NUM_GRAPHS, H], f32, tag="outs")
                    nc.scalar.activation(outs, pout, AF.Copy)
                    nc.sync.dma_start(out=out_d, in_=outs)

        if repeat > 1:
            with tc.For_i(0, repeat, 1) as _i:
                _kernel_body()
        else:
            _kernel_body()

    nc.finalize()
    return nc


_CACHE = {}


def _get_bass(e_cap):
    if e_cap not in _CACHE:
        _CACHE[e_cap] = _build_bass(e_cap)
    return _CACHE[e_cap]


def kernel(**inputs):
    from concourse.bass_utils import run_bass_kernel_spmd

    per_core, shared, e_cap = _host_preprocess(inputs)
    in_maps = [{**pc, **shared} for pc in per_core]
    nc = _get_bass(e_cap)
    res = run_bass_kernel_spmd(nc, in_maps, core_ids=list(range(NCORES)))
    out = np.zeros((NUM_GRAPHS, H), dtype=np.float32)
    for r in res.results:
        out += np.asarray(r["out"], dtype=np.float32)
    return out


# revision 17
# speedup vs baseline: 11.7863x; 2.6653x over previous
"""Trainium2 Bass kernel for nn_Arch7V3GraphEncoder (gnn_message_passing).

Strategy (graph/data parallel across 8 NeuronCores):
  - Canonical nodes are partitioned across the 8 cores. Because every edge is
    intra-subgraph (src and dst share e_sub) and subgraphs are node-local,
    each core runs the full 4-layer GIN stack on its shard with no
    communication; the final per-graph add-pool partials [32,128] are summed
    on the host.
  - Irregular gather/scatter is expressed as one-hot matmuls on the
    TensorEngine. Unlike the earlier revision (which shipped ~22 MB of
    host-built one-hot matrices per core and was transfer-bound), the host
    now sends only compact uint8 index arrays (~0.4 MB/core); the one-hot
    matrices are built on device:
      * column-wise one-hots (gather S, atom X, bond B): PE ones-broadcast of
        the index row into PSUM, then DVE is_equal against the partition iota;
      * row-wise one-hots (scatter D, graph G): DVE is_equal of a free-dim
        iota against a per-partition f32 index column.
    Invalid/empty slots carry index 200, which matches no iota lane and
    yields an all-zero one-hot column/row.
  - The (1+eps)*h term rides the MLP as an extra accumulating matmul; the
    bond-embedding matrix B is built once and staged through a DRAM scratch
    tile, streamed back per layer-group exactly like the old input path.
  - Pooling: subgraph masked-sum via a per-tile P1 one-hot matmul (built on
    device from the valid mask and a constant 128x16 pattern); softmax over
    log_probs on device; weighted reduce + canonical transposes + graph
    one-hot matmul produce the per-core [32,128] partial.
"""

import sys

sys.path.insert(0, "/opt/trn_rl_repo")

import numpy as np
import ml_dtypes

BF16 = ml_dtypes.bfloat16

# Problem constants (hardcoded per spec).
N_TOTAL = 4096
M_SUB = 4
K_NODES = 8
L_LAYERS = 4
H = 128
NUM_GRAPHS = 32
IN_CH = 119
EDGE_DIM = 8
S_ALL = N_TOTAL * M_SUB          # 16384 subgraphs
SK_ALL = S_ALL * K_NODES         # 131072 flat nodes
E_ALL = 12 * S_ALL               # 196608 edges
NCORES = 8
S_LOC = S_ALL // NCORES          # 2048 subgraphs / core
SK_LOC = SK_ALL // NCORES        # 16384 flat nodes / core
NT = SK_LOC // 128               # 128 tiles of 128 nodes
SG_T = 16                        # subgraphs per tile
NCAN_LOC = N_TOTAL // NCORES     # 512 canonical nodes / core
NQ = NCAN_LOC // 128             # 4 canonical chunks of 128
NG = NT // 4                     # 32 groups of 4 tiles (512 nodes)
EMPTY = 200                      # one-hot index that matches no lane


def _host_preprocess(inputs):
    """Compact integer index preprocessing -> small per-core uint8 arrays."""
    x_tokens = np.asarray(inputs["x_tokens"]).astype(np.int64)
    edge_tokens = np.asarray(inputs["edge_tokens"]).astype(np.int64)
    intra_ei = np.asarray(inputs["intra_ei"]).astype(np.int64)
    node_ids = np.asarray(inputs["node_ids"]).astype(np.int64)
    valid = np.asarray(inputs["valid"]).astype(bool)
    log_probs = np.asarray(inputs["log_probs"]).astype(np.float32)
    batch_graph = np.asarray(inputs["batch_graph"]).astype(np.int64)

    src, dst = intra_ei[0], intra_ei[1]
    e_sub = src // K_NODES
    assert np.array_equal(dst // K_NODES, e_sub), "edges must be intra-subgraph"

    core_of_e = e_sub // S_LOC
    tile_of_e = (e_sub % S_LOC) // SG_T
    key = core_of_e * NT + tile_of_e
    counts = np.bincount(key, minlength=NCORES * NT)
    e_cap = int(max(256, -(-counts.max() // 128) * 128))

    order = np.argsort(key, kind="stable")
    starts = np.zeros(NCORES * NT, dtype=np.int64)
    starts[1:] = np.cumsum(counts)[:-1]
    slot = np.empty(E_ALL, dtype=np.int64)
    slot[order] = np.arange(E_ALL) - starts[key[order]]

    ec = NT * e_cap
    nch = e_cap // 128
    nblk = ec // 128
    col = tile_of_e * e_cap + slot

    src_row = np.full((NCORES, 1, ec), EMPTY, dtype=np.uint8)
    tok_row = np.full((NCORES, 1, ec), EMPTY, dtype=np.uint8)
    dst_idx = np.full((NCORES, 128, nblk), EMPTY, dtype=np.uint8)
    src_row[core_of_e, 0, col] = np.where(valid[src], src % 128, EMPTY).astype(
        np.uint8
    )
    tok_row[core_of_e, 0, col] = edge_tokens.astype(np.uint8)
    dst_idx[core_of_e, slot % 128, tile_of_e * nch + slot // 128] = (
        dst % 128
    ).astype(np.uint8)

    j = np.arange(SK_ALL)
    j_core = j // SK_LOC
    j_loc = j % SK_LOC
    x_row = np.zeros((NCORES, 1, SK_LOC), dtype=np.uint8)
    x_row[j_core, 0, j_loc] = x_tokens.astype(np.uint8)

    vm = node_ids >= 0
    vmt = np.zeros((NCORES, 128, NT), dtype=np.uint8)
    vmt[j_core, j_loc % 128, j_loc // 128] = vm.astype(np.uint8)

    n = np.arange(N_TOTAL)
    bg = np.zeros((NCORES, 128, NQ), dtype=np.uint8)
    bg[n // NCAN_LOC, n % 128, (n % NCAN_LOC) // 128] = batch_graph.astype(np.uint8)

    p0 = np.zeros((128, SG_T), dtype=BF16)
    p0[np.arange(128), np.arange(128) // K_NODES] = np.asarray(1, dtype=BF16)

    lp = np.where(np.isfinite(log_probs), log_probs, 0.0).astype(np.float32)
    lp = lp.reshape(NCORES, 1, S_LOC)

    atom_emb = np.asarray(inputs["atom_emb"]).astype(np.float32)
    atom_pad = np.zeros((128, H), dtype=BF16)
    atom_pad[:IN_CH] = atom_emb.astype(BF16)

    b1 = np.asarray(inputs["mlp_b1"]).astype(np.float32)  # [L, H]
    b2 = np.asarray(inputs["mlp_b2"]).astype(np.float32)
    bvec = np.concatenate([b1.T, b2.T], axis=1)           # [H, 2L]
    sc = np.zeros((1, 8), dtype=np.float32)
    sc[0, :L_LAYERS] = np.asarray(inputs["eps"]).astype(np.float32)
    sc[0, L_LAYERS] = np.asarray(inputs["ht_alpha"]).astype(np.float32)[0]

    bf16_bytes = np.concatenate([
        p0.reshape(-1),
        atom_pad.reshape(-1),
        np.asarray(inputs["role_emb"]).astype(BF16).reshape(-1),
        np.asarray(inputs["bond_emb"]).astype(BF16).reshape(-1),
        np.asarray(inputs["mlp_w1"]).astype(BF16).reshape(-1),
        np.asarray(inputs["mlp_w2"]).astype(BF16).reshape(-1),
    ]).view(np.uint8)

    per_core = []
    for c in range(NCORES):
        u8_bytes = np.concatenate([
            src_row[c].reshape(-1), tok_row[c].reshape(-1),
            x_row[c].reshape(-1), dst_idx[c].reshape(-1),
            vmt[c].reshape(-1), bg[c].reshape(-1),
        ])
        f32_bytes = np.concatenate([
            lp[c].reshape(-1), bvec.reshape(-1), sc.reshape(-1)
        ]).astype(np.float32).view(np.uint8)
        blob = np.concatenate([u8_bytes, f32_bytes, bf16_bytes]).reshape(1, -1)
        per_core.append({"blob": np.ascontiguousarray(blob)})

    shared = {}
    return per_core, shared, e_cap


def _build_bass(e_cap, msg_bufs=4, pm_bufs=4, pz_bufs=1, pmlp_bufs=3, lay_bufs=2, repeat=1):
    import concourse.bass as bass
    import concourse.mybir as mybir
    from concourse import bacc
    from concourse.tile import TileContext

    f32 = mybir.dt.float32
    bf16 = mybir.dt.bfloat16
    u8 = mybir.dt.uint8
    i16 = mybir.dt.int16
    AF = mybir.ActivationFunctionType
    ALU = mybir.AluOpType
    AX = mybir.AxisListType

    ec = NT * e_cap
    nch = e_cap // 128  # edge chunks per tile
    nblk = ec // 128

    nc = bacc.Bacc("TRN2", target_bir_lowering=False, debug=False, num_devices=NCORES)

    def din(name, shape, dt):
        return nc.dram_tensor(name, shape, dt, kind="ExternalInput").ap()

    # single packed u8 blob; offsets in BYTES
    SRC_O = 0
    TOK_O = SRC_O + ec
    X_O = TOK_O + ec
    DST_O = X_O + SK_LOC
    VMT_O = DST_O + 128 * nblk
    BG_O = VMT_O + 128 * NT
    F32_BASE = BG_O + 128 * NQ
    LP_O = F32_BASE
    BVEC_O = LP_O + 4 * S_LOC
    SC_O = BVEC_O + 4 * H * 2 * L_LAYERS
    BF_BASE = SC_O + 4 * 8
    P0_O = BF_BASE
    ATOM_O = P0_O + 2 * 128 * SG_T
    ROLE_O = ATOM_O + 2 * 128 * H
    BOND_O = ROLE_O + 2 * 2 * H
    W1_O = BOND_O + 2 * 8 * H
    W2_O = W1_O + 2 * L_LAYERS * H * H
    BLOB_N = W2_O + 2 * L_LAYERS * H * H

    blob_d = din("blob", [1, BLOB_N], u8)

    def u8row(off, n):
        return blob_d[0:1, off : off + n]

    def u8m(off, p, c):
        return blob_d[0, off : off + p * c].rearrange("(p c) -> p c", p=p)

    def bfrow(off, n):
        return blob_d[0:1, off : off + 2 * n].bitcast(bf16)

    def bfm(off, p, c):
        return blob_d[0, off : off + 2 * p * c].bitcast(bf16).rearrange(
            "(p c) -> p c", p=p)

    def f32row(off, n):
        return blob_d[0:1, off : off + 4 * n].bitcast(f32)

    def f32m(off, p, c):
        return blob_d[0, off : off + 4 * p * c].bitcast(f32).rearrange(
            "(p c) -> p c", p=p)

    out_d = nc.dram_tensor("out", [NUM_GRAPHS, H], f32, kind="ExternalOutput").ap()

    with TileContext(nc) as tc:
        def _kernel_body():
            with (
                tc.tile_pool(name="persist", bufs=1) as pp,
                tc.tile_pool(name="dramp", bufs=2, space="DRAM") as dp,
            ):
                s_sb = pp.tile([128, ec], bf16, tag="s")
                hT = pp.tile([128, SK_LOC], bf16, tag="hT")
                h_nm = pp.tile([128, SK_LOC], bf16, tag="hnm")
                p1_sb = pp.tile([128, NT * SG_T], bf16, tag="p1")
                p0_sb = pp.tile([128, SG_T], bf16, tag="p0")
                g_sb = pp.tile([128, NQ * NUM_GRAPHS], bf16, tag="g")
                atom_sb = pp.tile([128, H], bf16, tag="atom")
                role_sb = pp.tile([2, H], bf16, tag="role")
                role1_sb = pp.tile([1, H], bf16, tag="role1")
                roleD = pp.tile([1, H], bf16, tag="roleD")
                role0T = pp.tile([128, 1], f32, tag="role0T")
                bond_sb = pp.tile([8, H], bf16, tag="bond")
                w1_sb = pp.tile([128, L_LAYERS * H], bf16, tag="w1")
                w2_sb = pp.tile([128, L_LAYERS * H], bf16, tag="w2")
                bvec_sb = pp.tile([128, 2 * L_LAYERS], f32, tag="bvec")
                eps_sb = pp.tile([1, L_LAYERS], f32, tag="eps")
                e1bc = pp.tile([128, L_LAYERS], f32, tag="e1bc")
                al_sb = pp.tile([1, 1], f32, tag="al")
                rc_sb = pp.tile([1, S_LOC], f32, tag="rc")
                lp_sb = pp.tile([1, S_LOC], f32, tag="lp")
                w_bc = pp.tile([128, S_LOC], f32, tag="wbc")
                rbc = pp.tile([128, S_LOC // M_SUB], f32, tag="rbc")
                id_sb = pp.tile([128, 128], bf16, tag="id")
                iota_f = pp.tile([128, 128], bf16, tag="iota_f")
                iota_p = pp.tile([128, 1], f32, tag="iota_p")
                ones_f = pp.tile([1, 128], f32, tag="ones_f")
                ones_c = pp.tile([128, 1], bf16, tag="ones_c")
                dst_f = pp.tile([128, nblk], f32, tag="dst_f")
                ir_row = pp.tile([1, 512], bf16, tag="ir_row")
                bscr = dp.tile([8, ec], bf16, tag="bscr")
                dscr = dp.tile([128, ec], bf16, tag="dscr")

                b1_sb = bvec_sb[:, 0:L_LAYERS]
                b2_sb = bvec_sb[:, L_LAYERS : 2 * L_LAYERS]

                # ---------------- constants / index prep ----------------
                nc.gpsimd.iota(iota_f, [[1, 128]], channel_multiplier=0,
                               allow_small_or_imprecise_dtypes=True)
                nc.gpsimd.iota(iota_p, [[1, 1]], channel_multiplier=1,
                               allow_small_or_imprecise_dtypes=True)
                nc.gpsimd.memset(ones_f, 1.0)
                nc.gpsimd.memset(ones_c, 1.0)

                nc.sync.dma_start(out=p0_sb, in_=bfm(P0_O, 128, SG_T))
                nc.sync.dma_start(out=atom_sb, in_=bfm(ATOM_O, 128, H))
                nc.sync.dma_start(out=role_sb, in_=bfm(ROLE_O, 2, H))
                nc.sync.dma_start(out=role1_sb, in_=bfrow(ROLE_O + 2 * H, H))
                nc.sync.dma_start(out=bond_sb, in_=bfm(BOND_O, 8, H))
                for l in range(L_LAYERS):
                    nc.sync.dma_start(out=w1_sb[:, l * H : (l + 1) * H], in_=bfm(W1_O + 2 * l * H * H, 128, H))
                    nc.sync.dma_start(out=w2_sb[:, l * H : (l + 1) * H], in_=bfm(W2_O + 2 * l * H * H, 128, H))
                nc.sync.dma_start(out=bvec_sb, in_=f32m(BVEC_O, H, 2 * L_LAYERS))
                nc.sync.dma_start(out=eps_sb, in_=f32row(SC_O, L_LAYERS))
                nc.sync.dma_start(out=al_sb, in_=f32row(SC_O + 4 * L_LAYERS, 1))
                nc.sync.dma_start(out=lp_sb, in_=f32row(LP_O, S_LOC))

                nc.vector.tensor_scalar(id_sb, iota_f, iota_p[:, 0:1], None,
                                        op0=ALU.is_equal)
                nc.vector.tensor_tensor(roleD, role1_sb, role_sb[0:1, :],
                                        ALU.subtract)

                with (
                    tc.tile_pool(name="prep_sb", bufs=3) as prep,
                    tc.tile_pool(name="prep_ps", bufs=1, space="PSUM") as prep_ps,
                ):
                    # role0T: transpose role_emb[0] into a [128,1] bias column
                    pr = prep_ps.tile([128, 2], bf16, tag="pr", bufs=1)
                    nc.tensor.transpose(pr, role_sb, id_sb[0:2, 0:2])
                    nc.vector.tensor_copy(role0T, pr[:, 0:1])

                    # (1+eps) broadcast [128, L]
                    pse = prep_ps.tile([128, L_LAYERS], f32, tag="pse", bufs=1)
                    nc.tensor.matmul(pse, lhsT=ones_f, rhs=eps_sb, start=True,
                                     stop=True)
                    nc.scalar.activation(e1bc, pse, AF.Copy, bias=1.0)

                    # isroot row: (j % 8 == 0), period 512
                    ir_i = prep.tile([1, 512], i16, tag="ir_i")
                    nc.gpsimd.iota(ir_i.rearrange("p (a b) -> p a b", b=K_NODES),
                                   [[0, 512 // K_NODES], [1, K_NODES]],
                                   channel_multiplier=0)
                    nc.vector.tensor_scalar(ir_row, ir_i, 0.0, None,
                                            op0=ALU.is_equal)

                    # graph one-hot [128, NQ*32] and dst_f conversion
                    bg_u8 = prep.tile([128, NQ], u8, tag="bg_u8", bufs=1)
                    nc.sync.dma_start(out=bg_u8, in_=u8m(BG_O, 128, NQ))
                    bg_f = prep.tile([128, NQ], f32, tag="bg_f", bufs=1)
                    nc.vector.tensor_copy(bg_f, bg_u8)
                    nc.vector.tensor_tensor(
                        g_sb.rearrange("p (q g) -> p q g", g=NUM_GRAPHS),
                        bg_f.rearrange("p (q x) -> p q x", x=1)
                            .broadcast_to([128, NQ, NUM_GRAPHS]),
                        iota_f[:, 0:NUM_GRAPHS]
                            .rearrange("p (x g) -> p x g", x=1)
                            .broadcast_to([128, NQ, NUM_GRAPHS]),
                        ALU.is_equal)

                    dst_u8 = prep.tile([128, nblk], u8, tag="dst_u8", bufs=1)
                    nc.sync.dma_start(out=dst_u8, in_=u8m(DST_O, 128, nblk))
                    nc.vector.tensor_copy(dst_f, dst_u8)
                    BPC = nblk // 16  # blocks per chunk
                    for k in range(16):
                        dchunk = prep.tile([128, BPC * 128], bf16, tag="dchunk",
                                           bufs=2)
                        nc.vector.tensor_tensor(
                            dchunk.rearrange("p (b c) -> p b c", c=128),
                            iota_f.rearrange("p (x c) -> p x c", x=1)
                                .broadcast_to([128, BPC, 128]),
                            dst_f[:, k * BPC : (k + 1) * BPC]
                                .rearrange("p (b x) -> p b x", x=1)
                                .broadcast_to([128, BPC, 128]),
                            ALU.is_equal)
                        nc.sync.dma_start(
                            out=dscr[:, k * BPC * 128 : (k + 1) * BPC * 128],
                            in_=dchunk)

                    # P1 pooling one-hot from valid mask
                    vmt_u8 = prep.tile([128, NT], u8, tag="vmt_u8", bufs=1)
                    nc.sync.dma_start(out=vmt_u8, in_=u8m(VMT_O, 128, NT))
                    vmt_f = prep.tile([128, NT], f32, tag="vmt_f", bufs=1)
                    nc.vector.tensor_copy(vmt_f, vmt_u8)
                    nc.vector.tensor_tensor(
                        p1_sb.rearrange("p (t s) -> p t s", s=SG_T),
                        vmt_f.rearrange("p (t x) -> p t x", x=1)
                            .broadcast_to([128, NT, SG_T]),
                        p0_sb.rearrange("p (x s) -> p x s", x=1)
                            .broadcast_to([128, NT, SG_T]),
                        ALU.mult)

                    # S gather one-hot [128, ec] + B bond one-hot -> DRAM scratch.
                    # The u8 index rows are partition-broadcast by DMA, then a
                    # single is_equal against the partition iota per chunk.
                    SCH = 4096
                    for k in range(ec // SCH):
                        sl = slice(k * SCH, (k + 1) * SCH)
                        sstage = prep.tile([128, SCH], u8, tag="sstage", bufs=2)
                        nc.sync.dma_start(
                            out=sstage,
                            in_=u8row(SRC_O + k * SCH, SCH).broadcast_to([128, SCH]))
                        nc.vector.tensor_scalar(s_sb[:, sl], sstage,
                                                iota_p[:, 0:1], None,
                                                op0=ALU.is_equal)

                        tstage = prep.tile([8, SCH], u8, tag="tstage", bufs=2)
                        nc.sync.dma_start(
                            out=tstage,
                            in_=u8row(TOK_O + k * SCH, SCH).broadcast_to([8, SCH]))
                        bt8 = prep.tile([8, SCH], bf16, tag="bt8", bufs=1)
                        nc.vector.tensor_scalar(bt8, tstage, iota_p[0:8, 0:1],
                                                None, op0=ALU.is_equal)
                        nc.sync.dma_start(out=bscr[:, sl], in_=bt8)

                # ---------------- embed ----------------
                with (
                    tc.tile_pool(name="emb_sb", bufs=3) as ep,
                    tc.tile_pool(name="emb_ps", bufs=2, space="PSUM") as epp,
                    tc.tile_pool(name="emb_ps2", bufs=4, space="PSUM") as epp2,
                ):
                    for q in range(SK_LOC // 512):
                        sl = slice(q * 512, (q + 1) * 512)
                        xstage = ep.tile([128, 512], u8, tag="xstage")
                        nc.sync.dma_start(
                            out=xstage,
                            in_=u8row(X_O + q * 512, 512).broadcast_to([128, 512]))
                        xoh = ep.tile([128, 512], bf16, tag="xoh")
                        nc.vector.tensor_scalar(xoh, xstage, iota_p[:, 0:1], None,
                                                op0=ALU.is_equal)
                        ps = epp.tile([128, 512], f32, tag="pse")
                        nc.tensor.matmul(ps, lhsT=atom_sb, rhs=xoh, start=True,
                                         stop=False)
                        nc.tensor.matmul(ps, lhsT=roleD, rhs=ir_row, start=False,
                                         stop=True)
                        nc.scalar.activation(hT[:, sl], ps, AF.Identity, bias=role0T)

                    # softmax weights + subgraph-count reciprocals
                    for q in range(S_LOC // 512):
                        sl = slice(q * 512, (q + 1) * 512)
                        psc = epp.tile([1, 512], f32, tag="psc", bufs=2)
                        nc.tensor.matmul(psc, lhsT=ones_c, rhs=p1_sb[:, sl],
                                         start=True, stop=True)
                        cmx = ep.tile([1, 512], f32, tag="cmx", bufs=2)
                        nc.vector.tensor_scalar_max(cmx, psc, 1.0)
                        nc.vector.reciprocal(rc_sb[:, sl], cmx)

                    st = ep.tile([1, S_LOC], f32, tag="st", bufs=1)
                    nc.vector.tensor_scalar(
                        st, lp_sb, al_sb[:, 0:1], -1.0, op0=ALU.mult, op1=ALU.mult
                    )
                    et = ep.tile([1, S_LOC], f32, tag="et", bufs=1)
                    nc.scalar.activation(et, st, AF.Exp)
                    s4 = ep.tile([1, S_LOC // M_SUB], f32, tag="s4", bufs=1)
                    nc.vector.tensor_reduce(
                        s4, et.rearrange("p (a b) -> p a b", b=M_SUB), AX.X, ALU.add
                    )
                    r4 = ep.tile([1, S_LOC // M_SUB], f32, tag="r4", bufs=1)
                    nc.vector.reciprocal(r4, s4)
                    wr = ep.tile([1, S_LOC], f32, tag="wr", bufs=1)
                    nc.vector.tensor_tensor(wr, et, rc_sb, ALU.mult)
                    for q in range(S_LOC // 512):
                        pw = epp.tile([128, 512], f32, tag="pse")
                        nc.tensor.matmul(
                            pw, lhsT=ones_f, rhs=wr[:, q * 512 : (q + 1) * 512],
                            start=True, stop=True,
                        )
                        nc.vector.tensor_copy(w_bc[:, q * 512 : (q + 1) * 512], pw)
                    pw = epp.tile([128, 512], f32, tag="pse")
                    nc.tensor.matmul(pw, lhsT=ones_f, rhs=r4, start=True, stop=True)
                    nc.vector.tensor_copy(rbc, pw[:, : S_LOC // M_SUB])
                    for q in range(NT // 4):
                        pt = epp2.tile([128, 512], bf16, tag="pt", bufs=2)
                        for tt in range(4):
                            t = q * 4 + tt
                            nc.tensor.transpose(
                                pt[:, tt * 128 : (tt + 1) * 128],
                                hT[:, t * 128 : (t + 1) * 128], id_sb)
                        if q % 2 == 0:
                            nc.vector.tensor_copy(h_nm[:, q * 512 : (q + 1) * 512], pt)
                        else:
                            nc.scalar.activation(h_nm[:, q * 512 : (q + 1) * 512], pt, AF.Copy)

                # ---------------- layers ----------------
                with (
                    tc.tile_pool(name="lay_sb", bufs=lay_bufs) as lp_sbuf,
                    tc.tile_pool(name="msg_sb", bufs=msg_bufs) as mp,
                    tc.tile_pool(name="ps_m", bufs=pm_bufs, space="PSUM") as pm,
                    tc.tile_pool(name="ps_z", bufs=pz_bufs, space="PSUM") as pz,
                    tc.tile_pool(name="ps_mlp", bufs=pmlp_bufs, space="PSUM") as pmlp,
                ):
                    for l in range(L_LAYERS):
                        w1_l = w1_sb[:, l * H : (l + 1) * H]
                        w2_l = w2_sb[:, l * H : (l + 1) * H]
                        for g in range(NG):
                            dt_ = lp_sbuf.tile([128, 4 * e_cap], bf16, tag="d")
                            bt_ = lp_sbuf.tile([8, 4 * e_cap], bf16, tag="b")
                            nc.sync.dma_start(
                                out=dt_, in_=dscr[:, g * 4 * e_cap : (g + 1) * 4 * e_cap]
                            )
                            nc.sync.dma_start(
                                out=bt_, in_=bscr[:, g * 4 * e_cap : (g + 1) * 4 * e_cap]
                            )
                            psz = pz.tile([128, 512], f32, tag="z")
                            for tp in range(2):
                                psm = pm.tile([128, 2 * e_cap], f32, tag="m")
                                for tt2 in range(2):
                                    tt = tp * 2 + tt2
                                    t = g * 4 + tt
                                    base = tt2 * e_cap
                                    for ch in range(nch):
                                        c0 = t * e_cap + ch * 128
                                        nc.tensor.matmul(
                                            psm[:, base + ch * 128 : base + (ch + 1) * 128],
                                            lhsT=s_sb[:, c0 : c0 + 128],
                                            rhs=h_nm[:, t * 128 : (t + 1) * 128],
                                            start=True,
                                            stop=False,
                                        )
                                        nc.tensor.matmul(
                                            psm[:, base + ch * 128 : base + (ch + 1) * 128],
                                            lhsT=bt_[:, tt * e_cap + ch * 128 : tt * e_cap + (ch + 1) * 128],
                                            rhs=bond_sb,
                                            start=False,
                                            stop=True,
                                        )
                                msg = mp.tile([128, 2 * e_cap], bf16, tag="msg")
                                if (g + tp) % 2 == 0:
                                    nc.scalar.activation(msg, psm, AF.Relu)
                                else:
                                    nc.vector.tensor_scalar_max(msg, psm, 0.0)
                                for tt2 in range(2):
                                    tt = tp * 2 + tt2
                                    for ch in range(nch):
                                        nc.tensor.matmul(
                                            psz[:, tt * 128 : (tt + 1) * 128],
                                            lhsT=msg[:, tt2 * e_cap + ch * 128 : tt2 * e_cap + (ch + 1) * 128],
                                            rhs=dt_[:, tt * e_cap + ch * 128 : tt * e_cap + (ch + 1) * 128],
                                            start=(ch == 0),
                                            stop=(ch == nch - 1),
                                        )
                            gsl = slice(g * 512, (g + 1) * 512)
                            zin = mp.tile([128, 512], bf16, tag="aggr")
                            nc.vector.scalar_tensor_tensor(
                                zin, hT[:, gsl], e1bc[:, l : l + 1], psz,
                                op0=ALU.mult, op1=ALU.add,
                            )
                            psy = pmlp.tile([128, 512], f32, tag="y")
                            nc.tensor.matmul(psy, lhsT=w1_l, rhs=zin, start=True, stop=True)
                            y1 = mp.tile([128, 512], bf16, tag="y1")
                            nc.scalar.activation(y1, psy, AF.Relu, bias=b1_sb[:, l : l + 1])
                            psz2 = pmlp.tile([128, 512], f32, tag="y")
                            nc.tensor.matmul(psz2, lhsT=w2_l, rhs=y1, start=True, stop=True)
                            nc.scalar.activation(
                                hT[:, gsl], psz2, AF.Identity, bias=b2_sb[:, l : l + 1]
                            )
                            ptr = pmlp.tile([128, 512], bf16, tag="y")
                            for tt in range(4):
                                t0 = g * 4 + tt
                                nc.tensor.transpose(
                                    ptr[:, tt * 128 : (tt + 1) * 128],
                                    hT[:, t0 * 128 : (t0 + 1) * 128], id_sb)
                            if g % 2 == 0:
                                nc.vector.tensor_copy(h_nm[:, gsl], ptr)
                            else:
                                nc.scalar.activation(h_nm[:, gsl], ptr, AF.Copy)

                # ---------------- pooling ----------------
                with (
                    tc.tile_pool(name="po_sb", bufs=1) as po,
                    tc.tile_pool(name="po_big", bufs=1) as pob,
                    tc.tile_pool(name="ps_hs", bufs=1, space="PSUM") as phs,
                    tc.tile_pool(name="ps_sm", bufs=2, space="PSUM") as psm_p,
                    tc.tile_pool(name="ps_o", bufs=2, space="PSUM") as pso,
                ):
                    hs = phs.tile([128, S_LOC], f32, tag="hs")
                    for t in range(NT):
                        nc.tensor.matmul(
                            hs[:, t * SG_T : (t + 1) * SG_T],
                            lhsT=h_nm[:, t * 128 : (t + 1) * 128],
                            rhs=p1_sb[:, t * SG_T : (t + 1) * SG_T],
                            start=True,
                            stop=True,
                        )
                    wt = pob.tile([128, S_LOC], f32, tag="wt")
                    nc.vector.tensor_tensor(wt, hs, w_bc, ALU.mult)
                    ndT = pob.tile([128, NCAN_LOC], f32, tag="ndT")
                    nc.vector.tensor_reduce(
                        ndT,
                        wt.rearrange("p (a b) -> p a b", b=M_SUB),
                        AX.X,
                        ALU.add,
                    )
                    ndTb = pob.tile([128, NCAN_LOC], bf16, tag="ndTb")
                    nc.vector.tensor_tensor(ndTb, ndT, rbc, ALU.mult)
                    pout = pso.tile([NUM_GRAPHS, H], f32, tag="po")
                    for q in range(NQ):
                        ptq = psm_p.tile([128, 128], bf16, tag="pw")
                        nc.tensor.transpose(ptq, ndTb[:, q * 128 : (q + 1) * 128], id_sb)
                        nnm = po.tile([128, 128], bf16, tag="nnm")
                        nc.vector.tensor_copy(nnm, ptq)
                        nc.tensor.matmul(
                            pout,
                            lhsT=g_sb[:, q * NUM_GRAPHS : (q + 1) * NUM_GRAPHS],
                            rhs=nnm,
                            start=(q == 0),
                            stop=(q == NQ - 1),
                        )
                    outs = po.tile([NUM_GRAPHS, H], f32, tag="outs")
                    nc.scalar.activation(outs, pout, AF.Copy)
                    nc.sync.dma_start(out=out_d, in_=outs)

        if repeat > 1:
            with tc.For_i(0, repeat, 1) as _i:
                _kernel_body()
        else:
            _kernel_body()

    nc.finalize()
    return nc


_CACHE = {}


def _get_bass(e_cap):
    if e_cap not in _CACHE:
        _CACHE[e_cap] = _build_bass(e_cap)
    return _CACHE[e_cap]


def kernel(**inputs):
    from concourse.bass_utils import run_bass_kernel_spmd

    per_core, shared, e_cap = _host_preprocess(inputs)
    in_maps = [{**pc, **shared} for pc in per_core]
    nc = _get_bass(e_cap)
    res = run_bass_kernel_spmd(nc, in_maps, core_ids=list(range(NCORES)))
    out = np.zeros((NUM_GRAPHS, H), dtype=np.float32)
    for r in res.results:
        out += np.asarray(r["out"], dtype=np.float32)
    return out
